# revision 20
# baseline (speedup 1.0000x reference)
"""TRN2 Bass kernel for 2-layer GAT + grouped softmax (nn_Actor_1881195675935).

8-core SPMD. Nodes sharded contiguously (12500/core, padded to 12544 = 98
tiles of 128); edges live with the owner of their dst node in an ELLPACK
layout (partition = dst node, free axis = edge slots, per-core relabeling
minimizes slot padding). Per layer: each core computes [h | a_src] rows for
its nodes (att vectors folded into the weights on host), AllGathers the
table, and fetches per-edge rows with dma_gather (4 SWDGE queues, 256B rows,
int16 indices into 4 subtables). Attention softmax runs on DVE/ACT over the
slot axis (a_dst is a per-partition scalar; slot pads point at a poisoned
row with a_src=-1e4 so exp()==0). The final `index`-grouped softmax uses
baked fp8 one-hot matmuls + a 64KB AllReduce.
"""

import sys

sys.path.insert(0, "/opt/trn_rl_repo")

import numpy as np
import ml_dtypes  # noqa: F401

N = 100000
NPC_REAL = 12500
NPC = 12544               # = 98 * 128
NT = 98
NC = 8
NSUB = 4
SUBROWS = 2 * NPC
WBASES = [0, 22528, 45056, 67584]  # gather window base positions
F_IN = 128
H1, C1 = 2, 16
OUT = 16
WSEG = 256
TROW = 64                 # table row f32 elements (256B)
PAD_AS = -1.0e4
PAD_IDX = 12543
PADPOS = [12543, 37631, 62719, 87807]  # a pad-node position inside each window


def _preprocess(x, edge_index, index, W1, att_src1, att_dst1, b1,
                W2, att_src2, att_dst2, b2):
    f32 = np.float32
    src = np.asarray(edge_index[0], dtype=np.int64)
    dst = np.asarray(edge_index[1], dtype=np.int64)
    loops = np.arange(N, dtype=np.int64)
    src = np.concatenate([src, loops]).astype(np.int64)
    dst = np.concatenate([dst, loops]).astype(np.int64)

    owner_dst = dst // NPC_REAL
    ldst = dst - owner_dst * NPC_REAL

    # window bases: 4 windows of 32768 positions with overlap; edges whose
    # src position falls in an overlap may go to either window.
    WBASE = np.array([0, 22528, 45056, 67584], dtype=np.int64)
    RB = np.array([0, 22528, 32768, 45056, 55296, 67584, 77824, 100352],
                  dtype=np.int64)

    # need positions first: provisional relabeling requires counts; do a
    # two-step: positions depend only on the per-core permutation, which we
    # compute from window profiles, which depend on positions... break the
    # loop: window regions are defined on *positions*, and within-core
    # relabeling permutes positions within one core's 12544-range. Region
    # boundaries (multiples of 22528/32768) do not align with core
    # boundaries (12544), so a node's region can change with relabeling.
    # Use original-order positions for region assignment: pos0(g) =
    # owner*NPC + old_local. Relabeling then permutes *within* the core
    # range; a src's position changes by < NPC which can cross a region
    # boundary. To keep indices exact we compute final positions first with
    # a degree-based permutation, then assign windows from *final*
    # positions.
    counts_deg = np.bincount(owner_dst * NPC + ldst, minlength=NC * NPC)
    counts_deg = counts_deg.reshape(NC, NPC)
    orders = np.zeros((NC, NPC), dtype=np.int64)
    # first pass permutation: by degree (refined below by window profile)
    for c in range(NC):
        orders[c] = np.argsort(-counts_deg[c], kind="stable")
    inv_orders = np.argsort(orders, axis=1)
    pos = np.zeros(N, dtype=np.int64)
    ar = np.arange(NPC_REAL)
    for c in range(NC):
        pos[c * NPC_REAL + ar] = c * NPC + inv_orders[c][ar]

    spos = pos[src]
    region = np.searchsorted(RB, spos, side="right") - 1     # 0..6
    nid = owner_dst * NPC + ldst
    rcnt = np.zeros((NC * NPC, 7), dtype=np.int64)
    np.add.at(rcnt, (nid, region), 1)
    Ccum = np.concatenate([np.zeros((NC * NPC, 1), np.int64),
                           np.cumsum(rcnt, axis=1)], axis=1)  # [n, 8]
    deg = Ccum[:, 7]
    b = np.zeros((NC * NPC, 3), dtype=np.int64)
    for j in range(3):
        tgt = ((j + 1) * deg + 3) // 4
        b[:, j] = np.clip(tgt, Ccum[:, 2 * j + 1], Ccum[:, 2 * j + 2])
    b = np.maximum.accumulate(b, axis=1)
    cuts = np.concatenate([np.zeros((NC * NPC, 1), np.int64), b,
                           deg[:, None]], axis=1)             # [n, 5]
    nW = np.diff(cuts, axis=1)                                # [n, 4]

    # refine relabeling by window profile, then recompute everything that
    # depends on position. Window counts of a node do not depend on its own
    # position (only on its neighbors'), so refining the permutation does
    # change *other* nodes' profiles; accept one iteration (profiles shift
    # by few edges) and recompute regions/cuts after re-permuting.
    def _cluster(prof):
        out = []
        def rec(ids):
            if len(ids) <= 128:
                out.append(ids)
                return
            sub = prof[ids]
            d = int(np.argmax(sub.max(0) - sub.min(0)))
            ids = ids[np.argsort(-sub[:, d], kind="stable")]
            left = (len(ids) // 128 // 2) * 128
            rec(ids[:left]); rec(ids[left:])
        rec(np.arange(len(prof)))
        return np.concatenate(out)

    def _slots_of(order, n):
        nn = n[order].reshape(NT, 128, NSUB)
        return int(nn.max(axis=1).sum())

    for c in range(NC):
        prof = nW[c * NPC:(c + 1) * NPC]
        cand1 = np.lexsort((-prof[:, 3], -prof[:, 2], -prof[:, 1], -prof[:, 0]))
        cand2 = _cluster(prof)
        best = cand1 if _slots_of(cand1, prof) <= _slots_of(cand2, prof) else cand2
        orders[c] = best
    inv_orders = np.argsort(orders, axis=1)
    for c in range(NC):
        pos[c * NPC_REAL + ar] = c * NPC + inv_orders[c][ar]

    spos = pos[src]
    region = np.searchsorted(RB, spos, side="right") - 1
    new_ldst = inv_orders[owner_dst, ldst]
    nid = owner_dst * NPC + new_ldst
    rcnt = np.zeros((NC * NPC, 7), dtype=np.int64)
    np.add.at(rcnt, (nid, region), 1)
    Ccum = np.concatenate([np.zeros((NC * NPC, 1), np.int64),
                           np.cumsum(rcnt, axis=1)], axis=1)
    deg = Ccum[:, 7]
    b = np.zeros((NC * NPC, 3), dtype=np.int64)
    for j in range(3):
        tgt = ((j + 1) * deg + 3) // 4
        b[:, j] = np.clip(tgt, Ccum[:, 2 * j + 1], Ccum[:, 2 * j + 2])
    b = np.maximum.accumulate(b, axis=1)
    cuts = np.concatenate([np.zeros((NC * NPC, 1), np.int64), b,
                           deg[:, None]], axis=1)
    nW = np.diff(cuts, axis=1)                                # [n, 4]

    ncounts = nW.reshape(NC, NPC, NSUB)
    S = ncounts.reshape(NC, NT, 128, NSUB).max(axis=(0, 2))   # [NT, NSUB]

    # edges sorted by (node, src position)
    eorder = np.lexsort((spos, nid))
    s_spos = spos[eorder]
    run_starts = np.zeros(NC * NPC + 1, dtype=np.int64)
    np.cumsum(np.bincount(nid, minlength=NC * NPC), out=run_starts[1:])

    # group pairs of tiles into one gather per window: fewer, larger SWDGE
    # desc-gen instructions (the gen rate is the kernel's floor).
    GT = 2
    NG = NT // GT
    GW = np.zeros((NG, NSUB), dtype=np.int64)   # group window widths
    for g in range(NG):
        GW[g] = S[GT * g:GT * (g + 1)].sum(axis=0)
    gwt = GW.sum(axis=1)
    gidx_off = np.concatenate([[0], np.cumsum(8 * gwt)]).astype(np.int64)
    IDXW = int(gidx_off[-1])

    def _block(c, t, w):
        sq = int(S[t, w])
        nodes = c * NPC + t * 128 + np.arange(128)
        r0n = run_starts[nodes]
        lo = cuts[nodes, w]
        nq = nW[nodes, w]
        i = np.arange(sq)[:, None]
        mask = i < nq[None, :]
        gi_ = np.minimum(r0n[None, :] + lo[None, :] + i, len(s_spos) - 1)
        padw = PADPOS[w] - int(WBASE[w])
        return np.where(mask, s_spos[gi_] - int(WBASE[w]), padw)  # [sq, 128]

    gidx = np.zeros((NC, 128, IDXW), dtype=np.int16)
    for c in range(NC):
        for g in range(NG):
            parts = []
            for w in range(NSUB):
                for t in range(GT * g, GT * (g + 1)):
                    if int(S[t, w]):
                        parts.append(_block(c, t, w))
            flat = np.concatenate(parts, axis=0)       # [gwt[g], 128]
            w16 = flat.reshape(-1, 16).T.astype(np.int16)
            gidx[c, :, gidx_off[g]:gidx_off[g + 1]] = np.tile(w16, (8, 1))

    W1 = np.asarray(W1, f32); W2 = np.asarray(W2, f32)
    as1 = np.asarray(att_src1, f32); ad1 = np.asarray(att_dst1, f32)
    as2 = np.asarray(att_src2, f32); ad2 = np.asarray(att_dst2, f32)
    vs1 = np.stack([W1[:, h * C1:(h + 1) * C1] @ as1[h] for h in range(H1)], 1)
    vd1 = np.stack([W1[:, h * C1:(h + 1) * C1] @ ad1[h] for h in range(H1)], 1)
    wcat1 = np.concatenate([W1, vs1, vd1], axis=1).astype(f32)
    vs2 = (W2 @ as2[0])[:, None]
    vd2 = (W2 @ ad2[0])[:, None]
    wcat2 = np.concatenate([W2, vs2, vd2], axis=1).astype(f32)

    x = np.asarray(x, f32)
    xT = np.zeros((NC, F_IN, NPC), dtype=f32)
    glb = np.zeros((NC, NPC), dtype=np.int64)
    real = np.zeros((NC, NPC), dtype=bool)
    for c in range(NC):
        ol = orders[c]
        is_real = ol < NPC_REAL
        g = np.where(is_real, c * NPC_REAL + np.minimum(ol, NPC_REAL - 1), 0)
        xT[c] = np.where(is_real[:, None], x[g], 0.0).astype(f32).T
        glb[c] = g
        real[c] = is_real

    index = np.asarray(index, np.int64)
    seg = np.zeros((NC, NPC), dtype=np.int64)
    g0 = np.zeros(NC, dtype=np.int64)
    for c in range(NC):
        seg[c] = np.where(real[c], index[glb[c]], 0)
        s = seg[c][real[c]]
        g0[c] = s.min()
        assert s.max() - s.min() < WSEG, "segment window exceeds WSEG"
    f8 = ml_dtypes.float8_e4m3
    # ohf[c]: [NT*128, 256]  (lhsT chunks along free); oht[c]: [NT*128, 256]
    ohf = np.zeros((NC, NT * 128, WSEG), dtype=f8)
    oht = np.zeros((NC, NT * 128, WSEG), dtype=f8)
    for c in range(NC):
        for t in range(NT):
            sl = seg[c, t * 128:(t + 1) * 128] - g0[c]
            m = real[c, t * 128:(t + 1) * 128]
            oh = np.zeros((128, WSEG), dtype=np.float32)
            oh[np.arange(128)[m], sl[m]] = 1.0
            ohf[c, t * 128:(t + 1) * 128] = oh.astype(f8)
            # bwd lhsT chunk k: [128 segs, 128 nodes] -> store as [128, 2*128]
            ohtk = np.concatenate([oh[:, :128].T, oh[:, 128:].T], axis=1)
            oht[c, t * 128:(t + 1) * 128] = ohtk.astype(f8)

    padfix = np.zeros((128, 3), dtype=f32)
    padfix[84:128, :] = PAD_AS

    sidx = np.zeros((NC, 128, 2), dtype=np.int32)
    for c in range(NC):
        for k in range(2):
            sidx[c, :, k] = g0[c] + k * 128 + np.arange(128)

    b1t = np.tile(np.asarray(b1, f32)[None, :], (128, 1)).astype(f32)
    b2t = np.tile(np.asarray(b2, f32)[None, :], (128, 1)).astype(f32)

    per_core = [{
        "xT": np.ascontiguousarray(xT[c]),
        "wcat1": wcat1, "wcat2": wcat2, "b1t": b1t, "b2t": b2t,
        "gidx": np.ascontiguousarray(gidx[c]),
        "padfix": padfix,
        "ohf": np.ascontiguousarray(ohf[c]),
        "oht": np.ascontiguousarray(oht[c]),
        "sidx": np.ascontiguousarray(sidx[c]),
    } for c in range(NC)]
    shared = {"S": S, "GT": GT, "NG": NG, "GW": GW, "gidx_off": gidx_off,
              "IDXW": IDXW}
    asm = {"glb": glb, "real": real}
    return shared, per_core, asm


def _build(shared):
    import concourse.bass as bass
    import concourse.bacc as bacc
    import concourse.tile as tile
    from concourse import mybir, library_config
    from concourse.masks import make_identity

    S = shared["S"]; IDXW = shared["IDXW"]
    GT = shared["GT"]; NG = shared["NG"]; GW = shared["GW"]
    gidx_off = shared["gidx_off"]
    f32 = mybir.dt.float32
    bf16 = mybir.dt.bfloat16
    f8 = mybir.dt.float8e4
    i16 = mybir.dt.int16
    AL = mybir.AluOpType
    EXP = mybir.ActivationFunctionType.Exp
    COPYF = mybir.ActivationFunctionType.Copy
    RELU = mybir.ActivationFunctionType.Relu
    IOA = bass.IndirectOffsetOnAxis

    nc = bacc.Bacc("TRN2", target_bir_lowering=False, debug=False,
                   num_devices=NC, num_swdge_queues=4)

    xT_ext = nc.dram_tensor("xT", [F_IN, NPC], f32, kind="ExternalInput")
    wcat1_ext = nc.dram_tensor("wcat1", [F_IN, 36], f32, kind="ExternalInput")
    wcat2_ext = nc.dram_tensor("wcat2", [32, 18], f32, kind="ExternalInput")
    b1_ext = nc.dram_tensor("b1t", [128, 32], f32, kind="ExternalInput")
    b2_ext = nc.dram_tensor("b2t", [128, 16], f32, kind="ExternalInput")
    gidx_ext = nc.dram_tensor("gidx", [128, IDXW], i16, kind="ExternalInput")
    ohf_ext = nc.dram_tensor("ohf", [NT * 128, WSEG], f8, kind="ExternalInput")
    oht_ext = nc.dram_tensor("oht", [NT * 128, WSEG], f8, kind="ExternalInput")
    sidx_ext = nc.dram_tensor("sidx", [128, 2], mybir.dt.int32, kind="ExternalInput")
    padfix_ext = nc.dram_tensor("padfix", [128, 3], f32, kind="ExternalInput")
    out_ext = nc.dram_tensor("out", [NPC, OUT], f32, kind="ExternalOutput")

    with tile.TileContext(nc) as tc:
        with (
            tc.tile_pool(name="dram", bufs=1, space="DRAM") as dr,
            tc.tile_pool(name="const", bufs=1) as cpool,
            tc.tile_pool(name="sbuf", bufs=4) as sb,
            tc.tile_pool(name="gat", bufs=2) as gp,
            tc.tile_pool(name="gia", bufs=6) as gia,
            tc.tile_pool(name="psum", bufs=2, space="PSUM") as pp,
            tc.tile_pool(name="psum2", bufs=1, space="PSUM") as pp2,
            tc.tile_pool(name="psum_seg", bufs=1, space="PSUM") as pseg,
            tc.tile_pool(name="res", bufs=1) as rp,
        ):
            tab1_loc = dr.tile([NPC, TROW], f32, name="tab1_loc")
            tab2_loc = dr.tile([NPC, TROW], f32, name="tab2_loc")
            tab1_full = dr.tile([NC * NPC, TROW], f32, name="tab1_full",
                                addr_space="Shared")
            tab2_full = dr.tile([NC * NPC, TROW], f32, name="tab2_full",
                                addr_space="Shared")
            s_loc = dr.tile([1280, OUT], f32, name="s_loc")
            s_red = dr.tile([1280, OUT], f32, name="s_red", addr_space="Shared")

            nc.gpsimd.load_library(library_config.mlp)

            ident = cpool.tile([128, 128], f32, name="ident")
            make_identity(nc, ident[:])
            wc1 = cpool.tile([F_IN, 36], f32, name="wc1")
            nc.sync.dma_start(out=wc1[:], in_=wcat1_ext[:, :])
            wc2 = cpool.tile([32, 18], f32, name="wc2")
            nc.sync.dma_start(out=wc2[:], in_=wcat2_ext[:, :])
            b1s = cpool.tile([128, 32], f32, name="b1s")
            nc.sync.dma_start(out=b1s[:], in_=b1_ext[:, :])
            b2s = cpool.tile([128, 16], f32, name="b2s")
            nc.sync.dma_start(out=b2s[:], in_=b2_ext[:, :])
            pfx = cpool.tile([128, 3], f32, name="pfx")
            nc.sync.dma_start(out=pfx[:], in_=padfix_ext[:, :])

            ad1_all = rp.tile([128, NT * 2], f32, name="ad1_all")
            ad2_all = rp.tile([128, NT], f32, name="ad2_all")
            e_all = rp.tile([128, NT * OUT], f32, name="e_all")
            ebf_all = rp.tile([128, NT * OUT], bf16, name="ebf_all")
            x2_all = rp.tile([128, NT * 32], f32, name="x2_all")
            hs_all = rp.tile([128, NT * 36], f32, name="hs_all")
            h2s_all = rp.tile([128, NT * 18], f32, name="h2s_all")
            fo_all = rp.tile([128, NT * OUT], f32, name="fo_all")

            c02 = cpool.tile([128, 1], f32, name="c02")
            nc.vector.memset(c02[:], 0.2)
            c30 = cpool.tile([128, 1], f32, name="c30")
            nc.vector.memset(c30[:], 1e-30)

            # ---- phase 0: table1 rows -------------------------------------
            for t in range(NT):
                xt = sb.tile([128, 128], f32, name=f"xt{t}", tag="xt")
                nc.sync.dma_start(out=xt[:], in_=xT_ext[:, t * 128:(t + 1) * 128])
                hp = pp.tile([128, 36], f32, name=f"hp{t}", tag="hp")
                nc.tensor.matmul(out=hp[:], lhsT=xt[:], rhs=wc1[:],
                                 start=True, stop=True)
                hs = hs_all[:, 36 * t:36 * (t + 1)]
                nc.vector.tensor_copy(out=hs, in_=hp[:])
                if t == NT - 1:
                    nc.vector.tensor_tensor(out=hs_all[:, 36 * t + 32:36 * t + 34],
                                            in0=hs_all[:, 36 * t + 32:36 * t + 34],
                                            in1=pfx[:, 0:2], op=AL.add)
                nc.vector.tensor_copy(out=ad1_all[:, 2 * t:2 * t + 2],
                                      in_=hs_all[:, 36 * t + 34:36 * t + 36])
            nc.sync.dma_start(
                out=tab1_loc.rearrange("(t p) e -> p t e", p=128)[:, :, 0:36],
                in_=hs_all[:].rearrange("p (t e) -> p t e", e=36))

            nc.gpsimd.collective_compute(
                "AllGather", AL.bypass, replica_groups=[list(range(NC))],
                ins=[tab1_loc.opt()], outs=[tab1_full.opt()])

            # ---- phase 1 (+fused table2 rows), grouped gathers ------------
            for g in range(NG):
                gbase = int(gidx_off[g])
                gwtot = int(GW[g].sum())
                gi = gia.tile([128, 8 * gwtot], i16, name=f"gi1_{g}", tag="gi")
                nc.sync.dma_start(out=gi[:],
                                  in_=gidx_ext[:, gbase:gbase + 8 * gwtot])
                gqs = []
                colg = 0
                for q in range(NSUB):
                    gwq = int(GW[g, q])
                    if gwq == 0:
                        gqs.append(None)
                        continue
                    gq = gp.tile([128, gwq, TROW], f32, name=f"g1_{g}q{q}",
                                 tag=f"g1q{q}")
                    nidx = 128 * gwq
                    nc.gpsimd.dma_gather(
                        gq[:], tab1_full[WBASES[q]:WBASES[q] + 32768, :],
                        gi[:, 8 * colg:8 * (colg + gwq)],
                        nidx, nidx, TROW, queue_num=q, single_packet=False)
                    gqs.append(gq)
                    colg += gwq
                for j in range(GT):
                    t = GT * g + j
                    wt = int(S[t].sum())
                    gxs = []
                    for q in range(NSUB):
                        sq = int(S[t, q])
                        if sq == 0 or gqs[q] is None:
                            gxs.append((None, 0, 0))
                            continue
                        ofs = int(S[GT * g:t, q].sum())
                        gxs.append((gqs[q], sq, ofs))
                    x2 = sb.tile([128, 32], f32, name=f"x2_{t}", tag="x2")
                    for h in range(H1):
                        z = sb.tile([128, wt], f32, name=f"z{t}h{h}", tag=f"z{h}")
                        zs = sb.tile([128, wt], f32, name=f"zs{t}h{h}",
                                     tag=f"zs{h}")
                        adc = ad1_all[:, 2 * t + h:2 * t + h + 1]
                        adb = adc.to_broadcast([128, wt])
                        co = 0
                        for gq, sq, ofs in gxs:
                            if gq is None:
                                continue
                            a_s = gq[:, ofs:ofs + sq, 32 + h:33 + h].rearrange(
                                "p w e -> p (w e)")
                            nc.vector.tensor_tensor(out=z[:, co:co + sq],
                                                    in0=a_s,
                                                    in1=adb[:, co:co + sq],
                                                    op=AL.add)
                            co += sq
                        nc.vector.tensor_tensor(
                            out=zs[:], in0=z[:],
                            in1=c02[:, 0:1].to_broadcast([128, wt]), op=AL.mult)
                        nc.vector.tensor_tensor(out=z[:], in0=z[:], in1=zs[:],
                                                op=AL.max)
                        ex = sb.tile([128, wt], f32, name=f"ex{t}h{h}",
                                     tag=f"ex{h}")
                        den = sb.tile([128, 1], f32, name=f"den{t}h{h}",
                                      tag="den")
                        nc.scalar.activation(out=ex[:], in_=z[:], func=EXP,
                                             accum_out=den[:, 0:1])
                        msg = sb.tile([128, wt, C1], f32, name=f"msg{t}h{h}",
                                      tag="msg")
                        co = 0
                        for gq, sq, ofs in gxs:
                            if gq is None:
                                continue
                            exb = ex[:, co:co + sq, None].to_broadcast(
                                [128, sq, C1])
                            nc.vector.tensor_tensor(
                                out=msg[:, co:co + sq, :],
                                in0=gq[:, ofs:ofs + sq, 16 * h:16 * h + 16],
                                in1=exb, op=AL.mult)
                            co += sq
                        num = sb.tile([128, C1], f32, name=f"num{t}h{h}",
                                      tag="num")
                        nc.vector.reduce_sum(
                            out=num[:], in_=msg[:].rearrange("p w e -> p e w"),
                            axis=mybir.AxisListType.X)
                        nc.vector.tensor_tensor(out=den[:], in0=den[:],
                                                in1=c30[:, 0:1], op=AL.max)
                        rcp = sb.tile([128, 1], f32, name=f"rcp{t}h{h}",
                                      tag="rcp")
                        nc.vector.reciprocal(out=rcp[:], in_=den[:])
                        nc.vector.tensor_tensor(
                            out=x2[:, 16 * h:16 * h + 16], in0=num[:],
                            in1=rcp[:, 0:1].to_broadcast([128, C1]), op=AL.mult)
                    nc.vector.tensor_tensor(out=x2[:], in0=x2[:], in1=b1s[:],
                                            op=AL.add)
                    nc.scalar.activation(out=x2_all[:, 32 * t:32 * (t + 1)],
                                         in_=x2[:], func=RELU)
                    # fused table2 row for tile t
                    x2tp = pp2.tile([32, 128], f32, name=f"x2tp{t}", tag="x2tp")
                    nc.tensor.transpose(out=x2tp[:],
                                        in_=x2_all[:, 32 * t:32 * (t + 1)],
                                        identity=ident[:])
                    x2ts = sb.tile([32, 128], f32, name=f"x2ts{t}", tag="x2ts")
                    nc.scalar.activation(out=x2ts[:], in_=x2tp[:], func=COPYF)
                    h2p = pp2.tile([128, 18], f32, name=f"h2p{t}", tag="h2p")
                    nc.tensor.matmul(out=h2p[:], lhsT=x2ts[:], rhs=wc2[:],
                                     start=True, stop=True)
                    h2s = h2s_all[:, 18 * t:18 * (t + 1)]
                    nc.scalar.activation(out=h2s, in_=h2p[:], func=COPYF)
                    if t == NT - 1:
                        nc.vector.tensor_tensor(
                            out=h2s_all[:, 18 * t + 16:18 * t + 17],
                            in0=h2s_all[:, 18 * t + 16:18 * t + 17],
                            in1=pfx[:, 2:3], op=AL.add)
                    nc.scalar.activation(out=ad2_all[:, t:t + 1],
                                         in_=h2s_all[:, 18 * t + 17:18 * t + 18],
                                         func=COPYF)
            nc.sync.dma_start(
                out=tab2_loc.rearrange("(t p) e -> p t e", p=128)[:, :, 0:18],
                in_=h2s_all[:].rearrange("p (t e) -> p t e", e=18))

            nc.gpsimd.collective_compute(
                "AllGather", AL.bypass, replica_groups=[list(range(NC))],
                ins=[tab2_loc.opt()], outs=[tab2_full.opt()])

            # ---- phase 3: layer-2 aggregation + exp + segment partials ----
            sp = [pseg.tile([128, OUT], f32, name=f"segp{k}") for k in range(2)]
            for g in range(NG):
                gbase = int(gidx_off[g])
                gwtot = int(GW[g].sum())
                gi = gia.tile([128, 8 * gwtot], i16, name=f"gi2_{g}", tag="gi")
                nc.sync.dma_start(out=gi[:],
                                  in_=gidx_ext[:, gbase:gbase + 8 * gwtot])
                gqs = []
                colg = 0
                for q in range(NSUB):
                    gwq = int(GW[g, q])
                    if gwq == 0:
                        gqs.append(None)
                        continue
                    gq = gp.tile([128, gwq, TROW], f32, name=f"g2_{g}q{q}",
                                 tag=f"g1q{q}")
                    nidx = 128 * gwq
                    nc.gpsimd.dma_gather(
                        gq[:], tab2_full[WBASES[q]:WBASES[q] + 32768, :],
                        gi[:, 8 * colg:8 * (colg + gwq)],
                        nidx, nidx, TROW, queue_num=q, single_packet=False)
                    gqs.append(gq)
                    colg += gwq
                for j in range(GT):
                    t = GT * g + j
                    wt = int(S[t].sum())
                    gxs = []
                    for q in range(NSUB):
                        sq = int(S[t, q])
                        if sq == 0 or gqs[q] is None:
                            gxs.append((None, 0, 0))
                            continue
                        ofs = int(S[GT * g:t, q].sum())
                        gxs.append((gqs[q], sq, ofs))
                    z = sb.tile([128, wt], f32, name=f"z2_{t}", tag="z0")
                    zs = sb.tile([128, wt], f32, name=f"zs2_{t}", tag="zs0")
                    adc = ad2_all[:, t:t + 1]
                    adb = adc.to_broadcast([128, wt])
                    co = 0
                    for gq, sq, ofs in gxs:
                        if gq is None:
                            continue
                        a_s = gq[:, ofs:ofs + sq, 16:17].rearrange(
                            "p w e -> p (w e)")
                        nc.vector.tensor_tensor(out=z[:, co:co + sq], in0=a_s,
                                                in1=adb[:, co:co + sq],
                                                op=AL.add)
                        co += sq
                    nc.vector.tensor_tensor(
                        out=zs[:], in0=z[:],
                        in1=c02[:, 0:1].to_broadcast([128, wt]), op=AL.mult)
                    nc.vector.tensor_tensor(out=z[:], in0=z[:], in1=zs[:],
                                            op=AL.max)
                    ex = sb.tile([128, wt], f32, name=f"ex2_{t}", tag="ex0")
                    den = sb.tile([128, 1], f32, name=f"den2_{t}", tag="den")
                    nc.scalar.activation(out=ex[:], in_=z[:], func=EXP,
                                         accum_out=den[:, 0:1])
                    msg = sb.tile([128, wt, OUT], f32, name=f"msg2_{t}",
                                  tag="msg")
                    co = 0
                    for gq, sq, ofs in gxs:
                        if gq is None:
                            continue
                        exb = ex[:, co:co + sq, None].to_broadcast(
                            [128, sq, OUT])
                        nc.vector.tensor_tensor(out=msg[:, co:co + sq, :],
                                                in0=gq[:, ofs:ofs + sq, 0:16],
                                                in1=exb, op=AL.mult)
                        co += sq
                    num = sb.tile([128, OUT], f32, name=f"num2_{t}", tag="num")
                    nc.vector.reduce_sum(
                        out=num[:], in_=msg[:].rearrange("p w e -> p e w"),
                        axis=mybir.AxisListType.X)
                    nc.vector.tensor_tensor(out=den[:], in0=den[:],
                                            in1=c30[:, 0:1], op=AL.max)
                    rcp = sb.tile([128, 1], f32, name=f"rcp2_{t}", tag="rcp")
                    nc.vector.reciprocal(out=rcp[:], in_=den[:])
                    o2 = sb.tile([128, OUT], f32, name=f"o2_{t}", tag="o2")
                    nc.vector.tensor_tensor(
                        out=o2[:], in0=num[:],
                        in1=rcp[:, 0:1].to_broadcast([128, OUT]), op=AL.mult)
                    nc.vector.tensor_tensor(out=o2[:], in0=o2[:], in1=b2s[:],
                                            op=AL.add)
                    nc.scalar.activation(out=e_all[:, OUT * t:OUT * (t + 1)],
                                         in_=o2[:], func=EXP)
                    nc.scalar.activation(out=ebf_all[:, OUT * t:OUT * (t + 1)],
                                         in_=e_all[:, OUT * t:OUT * (t + 1)],
                                         func=COPYF)
                    ohf_t = sb.tile([128, WSEG], f8, name=f"ohf{t}", tag="ohf")
                    nc.sync.dma_start(out=ohf_t[:],
                                      in_=ohf_ext[t * 128:(t + 1) * 128, :])
                    for k in range(2):
                        nc.tensor.matmul(out=sp[k][:],
                                         lhsT=ohf_t[:, k * 128:(k + 1) * 128],
                                         rhs=ebf_all[:, OUT * t:OUT * (t + 1)],
                                         start=(t == 0), stop=(t == NT - 1))

            # ---- phase 4: combine segment sums across cores ---------------
            zt = sb.tile([128, 160], f32, name="zt")
            nc.vector.memset(zt[:], 0.0)
            nc.sync.dma_start(
                out=s_loc.rearrange("(c p) f -> p c f", p=128),
                in_=zt[:].rearrange("p (c f) -> p c f", c=10))
            sxi = sb.tile([128, 2], mybir.dt.int32, name="sxi")
            nc.sync.dma_start(out=sxi[:], in_=sidx_ext[:, :])
            for k in range(2):
                spc = sb.tile([128, OUT], f32, name=f"spc{k}", tag="spc")
                nc.vector.tensor_copy(out=spc[:], in_=sp[k][:])
                nc.gpsimd.indirect_dma_start(
                    out=s_loc[:, :],
                    out_offset=IOA(ap=sxi[:, k:k + 1], axis=0),
                    in_=spc[:], in_offset=None)

            nc.gpsimd.collective_compute(
                "AllReduce", AL.add, replica_groups=[list(range(NC))],
                ins=[s_loc.opt()], outs=[s_red.opt()])

            sw = []
            for k in range(2):
                swf = sb.tile([128, OUT], f32, name=f"swf{k}", tag="swf")
                nc.gpsimd.indirect_dma_start(
                    out=swf[:], out_offset=None,
                    in_=s_red[:, :],
                    in_offset=IOA(ap=sxi[:, k:k + 1], axis=0))
                swb = rp.tile([128, OUT], bf16, name=f"sw{k}")
                nc.vector.tensor_copy(out=swb[:], in_=swf[:])
                sw.append(swb)

            # ---- phase 5: divide, write out -------------------------------
            for t in range(NT):
                oht_t = sb.tile([128, WSEG], f8, name=f"oht{t}", tag="oht")
                nc.sync.dma_start(out=oht_t[:],
                                  in_=oht_ext[t * 128:(t + 1) * 128, :])
                dp = pp.tile([128, OUT], f32, name=f"dp{t}", tag="dp")
                for k in range(2):
                    nc.tensor.matmul(out=dp[:],
                                     lhsT=oht_t[:, k * 128:(k + 1) * 128],
                                     rhs=sw[k][:], start=(k == 0), stop=(k == 1))
                dd = sb.tile([128, OUT], f32, name=f"dd{t}", tag="dd")
                nc.vector.tensor_tensor(out=dd[:], in0=dp[:],
                                        in1=c30[:, 0:1].to_broadcast([128, OUT]),
                                        op=AL.max)
                nc.vector.reciprocal(out=dd[:], in_=dd[:])
                nc.vector.tensor_tensor(out=fo_all[:, OUT * t:OUT * (t + 1)],
                                        in0=e_all[:, OUT * t:OUT * (t + 1)],
                                        in1=dd[:], op=AL.mult)
            nc.sync.dma_start(
                out=out_ext.rearrange("(t p) e -> p t e", p=128),
                in_=fo_all[:].rearrange("p (t e) -> p t e", e=OUT))

    nc.compile()
    return nc


def kernel_impl(inputs, trace=False):
    from concourse.bass_utils import run_bass_kernel_spmd
    shared, per_core, asm = _preprocess(**inputs)
    nc = _build(shared)
    res = run_bass_kernel_spmd(nc, per_core, core_ids=list(range(NC)),
                               trace=trace)
    out = np.zeros((N, OUT), dtype=np.float32)
    for c in range(NC):
        o = np.asarray(res.results[c]["out"])
        m = asm["real"][c]
        out[asm["glb"][c][m]] = o[m]
    return out, res


def kernel(**inputs):
    out, _ = kernel_impl(inputs, trace=False)
    return out



# revision 21
# speedup vs baseline: 1.2325x; 1.2325x over previous
"""TRN2 Bass kernel for 2-layer GAT + grouped softmax (nn_Actor_1881195675935).

8-core SPMD. Nodes sharded contiguously (12500/core, padded to 12544 = 98
tiles of 128); edges live with the owner of their dst node in an ELLPACK
layout (partition = dst node, free axis = edge slots, per-core relabeling
minimizes slot padding). Per layer: each core computes [h | a_src] rows for
its nodes (att vectors folded into the weights on host), AllGathers the
table, and fetches per-edge rows with dma_gather (4 SWDGE queues, 256B rows,
int16 indices into 4 subtables). Attention softmax runs on DVE/ACT over the
slot axis (a_dst is a per-partition scalar; slot pads point at a poisoned
row with a_src=-1e4 so exp()==0). The final `index`-grouped softmax uses
baked fp8 one-hot matmuls + a 64KB AllReduce.
"""

import sys

sys.path.insert(0, "/opt/trn_rl_repo")

import numpy as np
import ml_dtypes  # noqa: F401

N = 100000
NPC_REAL = 12500
NPC = 12544               # = 98 * 128
NT = 98
NC = 8
NSUB = 4
SUBROWS = 2 * NPC
WBASES = [0, 22528, 45056, 67584]  # gather window base positions
F_IN = 128
H1, C1 = 2, 16
OUT = 16
WSEG = 256
TROW = 64                 # table row f32 elements (256B)
PAD_AS = -1.0e4
PAD_IDX = 12543
PADPOS = [12543, 37631, 62719, 87807]  # a pad-node position inside each window


def _preprocess(x, edge_index, index, W1, att_src1, att_dst1, b1,
                W2, att_src2, att_dst2, b2):
    f32 = np.float32
    src = np.asarray(edge_index[0], dtype=np.int64)
    dst = np.asarray(edge_index[1], dtype=np.int64)
    loops = np.arange(N, dtype=np.int64)
    src = np.concatenate([src, loops]).astype(np.int64)
    dst = np.concatenate([dst, loops]).astype(np.int64)

    owner_dst = dst // NPC_REAL
    ldst = dst - owner_dst * NPC_REAL

    # window bases: 4 windows of 32768 positions with overlap; edges whose
    # src position falls in an overlap may go to either window.
    WBASE = np.array([0, 22528, 45056, 67584], dtype=np.int64)
    RB = np.array([0, 22528, 32768, 45056, 55296, 67584, 77824, 100352],
                  dtype=np.int64)

    # need positions first: provisional relabeling requires counts; do a
    # two-step: positions depend only on the per-core permutation, which we
    # compute from window profiles, which depend on positions... break the
    # loop: window regions are defined on *positions*, and within-core
    # relabeling permutes positions within one core's 12544-range. Region
    # boundaries (multiples of 22528/32768) do not align with core
    # boundaries (12544), so a node's region can change with relabeling.
    # Use original-order positions for region assignment: pos0(g) =
    # owner*NPC + old_local. Relabeling then permutes *within* the core
    # range; a src's position changes by < NPC which can cross a region
    # boundary. To keep indices exact we compute final positions first with
    # a degree-based permutation, then assign windows from *final*
    # positions.
    counts_deg = np.bincount(owner_dst * NPC + ldst, minlength=NC * NPC)
    counts_deg = counts_deg.reshape(NC, NPC)
    orders = np.zeros((NC, NPC), dtype=np.int64)
    # first pass permutation: by degree (refined below by window profile)
    for c in range(NC):
        orders[c] = np.argsort(-counts_deg[c], kind="stable")
    inv_orders = np.argsort(orders, axis=1)
    pos = np.zeros(N, dtype=np.int64)
    ar = np.arange(NPC_REAL)
    for c in range(NC):
        pos[c * NPC_REAL + ar] = c * NPC + inv_orders[c][ar]

    spos = pos[src]
    region = np.searchsorted(RB, spos, side="right") - 1     # 0..6
    nid = owner_dst * NPC + ldst
    rcnt = np.zeros((NC * NPC, 7), dtype=np.int64)
    np.add.at(rcnt, (nid, region), 1)
    Ccum = np.concatenate([np.zeros((NC * NPC, 1), np.int64),
                           np.cumsum(rcnt, axis=1)], axis=1)  # [n, 8]
    deg = Ccum[:, 7]
    b = np.zeros((NC * NPC, 3), dtype=np.int64)
    for j in range(3):
        tgt = ((j + 1) * deg + 3) // 4
        b[:, j] = np.clip(tgt, Ccum[:, 2 * j + 1], Ccum[:, 2 * j + 2])
    b = np.maximum.accumulate(b, axis=1)
    cuts = np.concatenate([np.zeros((NC * NPC, 1), np.int64), b,
                           deg[:, None]], axis=1)             # [n, 5]
    nW = np.diff(cuts, axis=1)                                # [n, 4]

    # refine relabeling by window profile, then recompute everything that
    # depends on position. Window counts of a node do not depend on its own
    # position (only on its neighbors'), so refining the permutation does
    # change *other* nodes' profiles; accept one iteration (profiles shift
    # by few edges) and recompute regions/cuts after re-permuting.
    def _cluster(prof):
        out = []
        def rec(ids):
            if len(ids) <= 128:
                out.append(ids)
                return
            sub = prof[ids]
            d = int(np.argmax(sub.max(0) - sub.min(0)))
            ids = ids[np.argsort(-sub[:, d], kind="stable")]
            left = (len(ids) // 128 // 2) * 128
            rec(ids[:left]); rec(ids[left:])
        rec(np.arange(len(prof)))
        return np.concatenate(out)

    def _slots_of(order, n):
        nn = n[order].reshape(NT, 128, NSUB)
        return int(nn.max(axis=1).sum())

    for c in range(NC):
        prof = nW[c * NPC:(c + 1) * NPC]
        cand1 = np.lexsort((-prof[:, 3], -prof[:, 2], -prof[:, 1], -prof[:, 0]))
        cand2 = _cluster(prof)
        best = cand1 if _slots_of(cand1, prof) <= _slots_of(cand2, prof) else cand2
        orders[c] = best
    inv_orders = np.argsort(orders, axis=1)
    for c in range(NC):
        pos[c * NPC_REAL + ar] = c * NPC + inv_orders[c][ar]

    spos = pos[src]
    region = np.searchsorted(RB, spos, side="right") - 1
    new_ldst = inv_orders[owner_dst, ldst]
    nid = owner_dst * NPC + new_ldst
    rcnt = np.zeros((NC * NPC, 7), dtype=np.int64)
    np.add.at(rcnt, (nid, region), 1)
    Ccum = np.concatenate([np.zeros((NC * NPC, 1), np.int64),
                           np.cumsum(rcnt, axis=1)], axis=1)
    deg = Ccum[:, 7]
    b = np.zeros((NC * NPC, 3), dtype=np.int64)
    for j in range(3):
        tgt = ((j + 1) * deg + 3) // 4
        b[:, j] = np.clip(tgt, Ccum[:, 2 * j + 1], Ccum[:, 2 * j + 2])
    b = np.maximum.accumulate(b, axis=1)
    cuts = np.concatenate([np.zeros((NC * NPC, 1), np.int64), b,
                           deg[:, None]], axis=1)
    nW = np.diff(cuts, axis=1)                                # [n, 4]

    ncounts = nW.reshape(NC, NPC, NSUB)
    S = ncounts.reshape(NC, NT, 128, NSUB).max(axis=(0, 2))   # [NT, NSUB]

    # edges sorted by (node, src position)
    eorder = np.lexsort((spos, nid))
    s_spos = spos[eorder]
    run_starts = np.zeros(NC * NPC + 1, dtype=np.int64)
    np.cumsum(np.bincount(nid, minlength=NC * NPC), out=run_starts[1:])

    # group pairs of tiles into one gather per window: fewer, larger SWDGE
    # desc-gen instructions (the gen rate is the kernel's floor).
    GT = 1
    NG = NT // GT
    GW = np.zeros((NG, NSUB), dtype=np.int64)   # group window widths
    for g in range(NG):
        GW[g] = S[GT * g:GT * (g + 1)].sum(axis=0)
    gwt = GW.sum(axis=1)
    gidx_off = np.concatenate([[0], np.cumsum(8 * gwt)]).astype(np.int64)
    IDXW = int(gidx_off[-1])

    def _block(c, t, w):
        sq = int(S[t, w])
        nodes = c * NPC + t * 128 + np.arange(128)
        r0n = run_starts[nodes]
        lo = cuts[nodes, w]
        nq = nW[nodes, w]
        i = np.arange(sq)[:, None]
        mask = i < nq[None, :]
        gi_ = np.minimum(r0n[None, :] + lo[None, :] + i, len(s_spos) - 1)
        padw = PADPOS[w] - int(WBASE[w])
        return np.where(mask, s_spos[gi_] - int(WBASE[w]), padw)  # [sq, 128]

    gidx = np.zeros((NC, 128, IDXW), dtype=np.int16)
    for c in range(NC):
        for g in range(NG):
            parts = []
            for w in range(NSUB):
                for t in range(GT * g, GT * (g + 1)):
                    if int(S[t, w]):
                        parts.append(_block(c, t, w))
            flat = np.concatenate(parts, axis=0)       # [gwt[g], 128]
            w16 = flat.reshape(-1, 16).T.astype(np.int16)
            gidx[c, :, gidx_off[g]:gidx_off[g + 1]] = np.tile(w16, (8, 1))

    W1 = np.asarray(W1, f32); W2 = np.asarray(W2, f32)
    as1 = np.asarray(att_src1, f32); ad1 = np.asarray(att_dst1, f32)
    as2 = np.asarray(att_src2, f32); ad2 = np.asarray(att_dst2, f32)
    vs1 = np.stack([W1[:, h * C1:(h + 1) * C1] @ as1[h] for h in range(H1)], 1)
    vd1 = np.stack([W1[:, h * C1:(h + 1) * C1] @ ad1[h] for h in range(H1)], 1)
    wcat1 = np.concatenate([W1, vs1, vd1], axis=1).astype(f32)
    vs2 = (W2 @ as2[0])[:, None]
    vd2 = (W2 @ ad2[0])[:, None]
    wcat2 = np.concatenate([W2, vs2, vd2], axis=1).astype(f32)

    x = np.asarray(x, f32)
    xT = np.zeros((NC, F_IN, NPC), dtype=f32)
    glb = np.zeros((NC, NPC), dtype=np.int64)
    real = np.zeros((NC, NPC), dtype=bool)
    for c in range(NC):
        ol = orders[c]
        is_real = ol < NPC_REAL
        g = np.where(is_real, c * NPC_REAL + np.minimum(ol, NPC_REAL - 1), 0)
        xT[c] = np.where(is_real[:, None], x[g], 0.0).astype(f32).T
        glb[c] = g
        real[c] = is_real

    index = np.asarray(index, np.int64)
    seg = np.zeros((NC, NPC), dtype=np.int64)
    g0 = np.zeros(NC, dtype=np.int64)
    for c in range(NC):
        seg[c] = np.where(real[c], index[glb[c]], 0)
        s = seg[c][real[c]]
        g0[c] = s.min()
        assert s.max() - s.min() < WSEG, "segment window exceeds WSEG"
    f8 = ml_dtypes.float8_e4m3
    # ohf[c]: [NT*128, 256]  (lhsT chunks along free); oht[c]: [NT*128, 256]
    ohf = np.zeros((NC, NT * 128, WSEG), dtype=f8)
    oht = np.zeros((NC, NT * 128, WSEG), dtype=f8)
    for c in range(NC):
        for t in range(NT):
            sl = seg[c, t * 128:(t + 1) * 128] - g0[c]
            m = real[c, t * 128:(t + 1) * 128]
            oh = np.zeros((128, WSEG), dtype=np.float32)
            oh[np.arange(128)[m], sl[m]] = 1.0
            ohf[c, t * 128:(t + 1) * 128] = oh.astype(f8)
            # bwd lhsT chunk k: [128 segs, 128 nodes] -> store as [128, 2*128]
            ohtk = np.concatenate([oh[:, :128].T, oh[:, 128:].T], axis=1)
            oht[c, t * 128:(t + 1) * 128] = ohtk.astype(f8)

    padfix = np.zeros((128, 3), dtype=f32)
    padfix[84:128, :] = PAD_AS

    sidx = np.zeros((NC, 128, 2), dtype=np.int32)
    for c in range(NC):
        for k in range(2):
            sidx[c, :, k] = g0[c] + k * 128 + np.arange(128)

    b1t = np.tile(np.asarray(b1, f32)[None, :], (128, 1)).astype(f32)
    b2t = np.tile(np.asarray(b2, f32)[None, :], (128, 1)).astype(f32)

    per_core = [{
        "xT": np.ascontiguousarray(xT[c]),
        "wcat1": wcat1, "wcat2": wcat2, "b1t": b1t, "b2t": b2t,
        "gidx": np.ascontiguousarray(gidx[c]),
        "padfix": padfix,
        "ohf": np.ascontiguousarray(ohf[c]),
        "oht": np.ascontiguousarray(oht[c]),
        "sidx": np.ascontiguousarray(sidx[c]),
    } for c in range(NC)]
    shared = {"S": S, "GT": GT, "NG": NG, "GW": GW, "gidx_off": gidx_off,
              "IDXW": IDXW}
    asm = {"glb": glb, "real": real}
    return shared, per_core, asm


def _build(shared):
    import concourse.bass as bass
    import concourse.bacc as bacc
    import concourse.tile as tile
    from concourse import mybir, library_config
    from concourse.masks import make_identity

    S = shared["S"]; IDXW = shared["IDXW"]
    GT = shared["GT"]; NG = shared["NG"]; GW = shared["GW"]
    gidx_off = shared["gidx_off"]
    f32 = mybir.dt.float32
    bf16 = mybir.dt.bfloat16
    f8 = mybir.dt.float8e4
    i16 = mybir.dt.int16
    AL = mybir.AluOpType
    EXP = mybir.ActivationFunctionType.Exp
    COPYF = mybir.ActivationFunctionType.Copy
    RELU = mybir.ActivationFunctionType.Relu
    IOA = bass.IndirectOffsetOnAxis

    nc = bacc.Bacc("TRN2", target_bir_lowering=False, debug=False,
                   num_devices=NC, num_swdge_queues=4)

    xT_ext = nc.dram_tensor("xT", [F_IN, NPC], f32, kind="ExternalInput")
    wcat1_ext = nc.dram_tensor("wcat1", [F_IN, 36], f32, kind="ExternalInput")
    wcat2_ext = nc.dram_tensor("wcat2", [32, 18], f32, kind="ExternalInput")
    b1_ext = nc.dram_tensor("b1t", [128, 32], f32, kind="ExternalInput")
    b2_ext = nc.dram_tensor("b2t", [128, 16], f32, kind="ExternalInput")
    gidx_ext = nc.dram_tensor("gidx", [128, IDXW], i16, kind="ExternalInput")
    ohf_ext = nc.dram_tensor("ohf", [NT * 128, WSEG], f8, kind="ExternalInput")
    oht_ext = nc.dram_tensor("oht", [NT * 128, WSEG], f8, kind="ExternalInput")
    sidx_ext = nc.dram_tensor("sidx", [128, 2], mybir.dt.int32, kind="ExternalInput")
    padfix_ext = nc.dram_tensor("padfix", [128, 3], f32, kind="ExternalInput")
    out_ext = nc.dram_tensor("out", [NPC, OUT], f32, kind="ExternalOutput")

    with tile.TileContext(nc) as tc:
        with (
            tc.tile_pool(name="dram", bufs=1, space="DRAM") as dr,
            tc.tile_pool(name="const", bufs=1) as cpool,
            tc.tile_pool(name="sbuf", bufs=4) as sb,
            tc.tile_pool(name="gat", bufs=4) as gp,
            tc.tile_pool(name="gia", bufs=6) as gia,
            tc.tile_pool(name="psum", bufs=2, space="PSUM") as pp,
            tc.tile_pool(name="psum2", bufs=1, space="PSUM") as pp2,
            tc.tile_pool(name="psum_seg", bufs=1, space="PSUM") as pseg,
            tc.tile_pool(name="res", bufs=1) as rp,
        ):
            tab1_loc = dr.tile([NPC, TROW], f32, name="tab1_loc")
            tab2_loc = dr.tile([NPC, TROW], f32, name="tab2_loc")
            tab1_full = dr.tile([NC * NPC, TROW], f32, name="tab1_full",
                                addr_space="Shared")
            tab2_full = dr.tile([NC * NPC, TROW], f32, name="tab2_full",
                                addr_space="Shared")
            s_loc = dr.tile([1280, OUT], f32, name="s_loc")
            s_red = dr.tile([1280, OUT], f32, name="s_red", addr_space="Shared")

            nc.gpsimd.load_library(library_config.mlp)

            ident = cpool.tile([128, 128], f32, name="ident")
            make_identity(nc, ident[:])
            wc1 = cpool.tile([F_IN, 36], f32, name="wc1")
            nc.sync.dma_start(out=wc1[:], in_=wcat1_ext[:, :])
            wc2 = cpool.tile([32, 18], f32, name="wc2")
            nc.sync.dma_start(out=wc2[:], in_=wcat2_ext[:, :])
            b1s = cpool.tile([128, 32], f32, name="b1s")
            nc.sync.dma_start(out=b1s[:], in_=b1_ext[:, :])
            b2s = cpool.tile([128, 16], f32, name="b2s")
            nc.sync.dma_start(out=b2s[:], in_=b2_ext[:, :])
            pfx = cpool.tile([128, 3], f32, name="pfx")
            nc.sync.dma_start(out=pfx[:], in_=padfix_ext[:, :])

            ad1_all = rp.tile([128, NT * 2], f32, name="ad1_all")
            ad2_all = rp.tile([128, NT], f32, name="ad2_all")
            e_all = rp.tile([128, NT * OUT], f32, name="e_all")
            ebf_all = rp.tile([128, NT * OUT], bf16, name="ebf_all")
            x2_all = rp.tile([128, NT * 32], f32, name="x2_all")
            hs_all = rp.tile([128, NT * 36], f32, name="hs_all")
            h2s_all = rp.tile([128, NT * 18], f32, name="h2s_all")
            fo_all = rp.tile([128, NT * OUT], f32, name="fo_all")

            c02 = cpool.tile([128, 1], f32, name="c02")
            nc.vector.memset(c02[:], 0.2)
            c30 = cpool.tile([128, 1], f32, name="c30")
            nc.vector.memset(c30[:], 1e-30)

            # ---- phase 0: table1 rows -------------------------------------
            for t in range(NT):
                xt = sb.tile([128, 128], f32, name=f"xt{t}", tag="xt")
                nc.sync.dma_start(out=xt[:], in_=xT_ext[:, t * 128:(t + 1) * 128])
                hp = pp.tile([128, 36], f32, name=f"hp{t}", tag="hp")
                nc.tensor.matmul(out=hp[:], lhsT=xt[:], rhs=wc1[:],
                                 start=True, stop=True)
                hs = hs_all[:, 36 * t:36 * (t + 1)]
                nc.vector.tensor_copy(out=hs, in_=hp[:])
                if t == NT - 1:
                    nc.vector.tensor_tensor(out=hs_all[:, 36 * t + 32:36 * t + 34],
                                            in0=hs_all[:, 36 * t + 32:36 * t + 34],
                                            in1=pfx[:, 0:2], op=AL.add)
                nc.vector.tensor_copy(out=ad1_all[:, 2 * t:2 * t + 2],
                                      in_=hs_all[:, 36 * t + 34:36 * t + 36])
            nc.sync.dma_start(
                out=tab1_loc.rearrange("(t p) e -> p t e", p=128)[:, :, 0:36],
                in_=hs_all[:].rearrange("p (t e) -> p t e", e=36))

            nc.gpsimd.collective_compute(
                "AllGather", AL.bypass, replica_groups=[list(range(NC))],
                ins=[tab1_loc.opt()], outs=[tab1_full.opt()])

            # ---- phase 1 (+fused table2 rows), grouped gathers ------------
            for g in range(NG):
                gbase = int(gidx_off[g])
                gwtot = int(GW[g].sum())
                gi = gia.tile([128, 8 * gwtot], i16, name=f"gi1_{g}", tag="gi")
                nc.sync.dma_start(out=gi[:],
                                  in_=gidx_ext[:, gbase:gbase + 8 * gwtot])
                gqs = []
                colg = 0
                for q in range(NSUB):
                    gwq = int(GW[g, q])
                    if gwq == 0:
                        gqs.append(None)
                        continue
                    gq = gp.tile([128, gwq, TROW], f32, name=f"g1_{g}q{q}",
                                 tag=f"g1q{q}")
                    nidx = 128 * gwq
                    nc.gpsimd.dma_gather(
                        gq[:], tab1_full[WBASES[q]:WBASES[q] + 32768, :],
                        gi[:, 8 * colg:8 * (colg + gwq)],
                        nidx, nidx, TROW, queue_num=q, single_packet=False)
                    gqs.append(gq)
                    colg += gwq
                for j in range(GT):
                    t = GT * g + j
                    wt = int(S[t].sum())
                    gxs = []
                    for q in range(NSUB):
                        sq = int(S[t, q])
                        if sq == 0 or gqs[q] is None:
                            gxs.append((None, 0, 0))
                            continue
                        ofs = int(S[GT * g:t, q].sum())
                        gxs.append((gqs[q], sq, ofs))
                    x2 = sb.tile([128, 32], f32, name=f"x2_{t}", tag="x2")
                    for h in range(H1):
                        z = sb.tile([128, wt], f32, name=f"z{t}h{h}", tag=f"z{h}")
                        zs = sb.tile([128, wt], f32, name=f"zs{t}h{h}",
                                     tag=f"zs{h}")
                        adc = ad1_all[:, 2 * t + h:2 * t + h + 1]
                        adb = adc.to_broadcast([128, wt])
                        co = 0
                        for gq, sq, ofs in gxs:
                            if gq is None:
                                continue
                            a_s = gq[:, ofs:ofs + sq, 32 + h:33 + h].rearrange(
                                "p w e -> p (w e)")
                            nc.vector.tensor_tensor(out=z[:, co:co + sq],
                                                    in0=a_s,
                                                    in1=adb[:, co:co + sq],
                                                    op=AL.add)
                            co += sq
                        nc.vector.tensor_tensor(
                            out=zs[:], in0=z[:],
                            in1=c02[:, 0:1].to_broadcast([128, wt]), op=AL.mult)
                        nc.vector.tensor_tensor(out=z[:], in0=z[:], in1=zs[:],
                                                op=AL.max)
                        ex = sb.tile([128, wt], f32, name=f"ex{t}h{h}",
                                     tag=f"ex{h}")
                        den = sb.tile([128, 1], f32, name=f"den{t}h{h}",
                                      tag="den")
                        nc.scalar.activation(out=ex[:], in_=z[:], func=EXP,
                                             accum_out=den[:, 0:1])
                        msg = sb.tile([128, wt, C1], f32, name=f"msg{t}h{h}",
                                      tag="msg")
                        co = 0
                        for gq, sq, ofs in gxs:
                            if gq is None:
                                continue
                            exb = ex[:, co:co + sq, None].to_broadcast(
                                [128, sq, C1])
                            nc.vector.tensor_tensor(
                                out=msg[:, co:co + sq, :],
                                in0=gq[:, ofs:ofs + sq, 16 * h:16 * h + 16],
                                in1=exb, op=AL.mult)
                            co += sq
                        num = sb.tile([128, C1], f32, name=f"num{t}h{h}",
                                      tag="num")
                        nc.vector.reduce_sum(
                            out=num[:], in_=msg[:].rearrange("p w e -> p e w"),
                            axis=mybir.AxisListType.X)
                        nc.vector.tensor_tensor(out=den[:], in0=den[:],
                                                in1=c30[:, 0:1], op=AL.max)
                        rcp = sb.tile([128, 1], f32, name=f"rcp{t}h{h}",
                                      tag="rcp")
                        nc.vector.reciprocal(out=rcp[:], in_=den[:])
                        nc.vector.tensor_tensor(
                            out=x2[:, 16 * h:16 * h + 16], in0=num[:],
                            in1=rcp[:, 0:1].to_broadcast([128, C1]), op=AL.mult)
                    nc.vector.tensor_tensor(out=x2[:], in0=x2[:], in1=b1s[:],
                                            op=AL.add)
                    nc.scalar.activation(out=x2_all[:, 32 * t:32 * (t + 1)],
                                         in_=x2[:], func=RELU)
                    # fused table2 row for tile t
                    x2tp = pp2.tile([32, 128], f32, name=f"x2tp{t}", tag="x2tp")
                    nc.tensor.transpose(out=x2tp[:],
                                        in_=x2_all[:, 32 * t:32 * (t + 1)],
                                        identity=ident[:])
                    x2ts = sb.tile([32, 128], f32, name=f"x2ts{t}", tag="x2ts")
                    nc.scalar.activation(out=x2ts[:], in_=x2tp[:], func=COPYF)
                    h2p = pp2.tile([128, 18], f32, name=f"h2p{t}", tag="h2p")
                    nc.tensor.matmul(out=h2p[:], lhsT=x2ts[:], rhs=wc2[:],
                                     start=True, stop=True)
                    h2s = h2s_all[:, 18 * t:18 * (t + 1)]
                    nc.scalar.activation(out=h2s, in_=h2p[:], func=COPYF)
                    if t == NT - 1:
                        nc.vector.tensor_tensor(
                            out=h2s_all[:, 18 * t + 16:18 * t + 17],
                            in0=h2s_all[:, 18 * t + 16:18 * t + 17],
                            in1=pfx[:, 2:3], op=AL.add)
                    nc.scalar.activation(out=ad2_all[:, t:t + 1],
                                         in_=h2s_all[:, 18 * t + 17:18 * t + 18],
                                         func=COPYF)
            nc.sync.dma_start(
                out=tab2_loc.rearrange("(t p) e -> p t e", p=128)[:, :, 0:18],
                in_=h2s_all[:].rearrange("p (t e) -> p t e", e=18))

            nc.gpsimd.collective_compute(
                "AllGather", AL.bypass, replica_groups=[list(range(NC))],
                ins=[tab2_loc.opt()], outs=[tab2_full.opt()])

            # ---- phase 3: layer-2 aggregation + exp + segment partials ----
            sp = [pseg.tile([128, OUT], f32, name=f"segp{k}") for k in range(2)]
            for g in range(NG):
                gbase = int(gidx_off[g])
                gwtot = int(GW[g].sum())
                gi = gia.tile([128, 8 * gwtot], i16, name=f"gi2_{g}", tag="gi")
                nc.sync.dma_start(out=gi[:],
                                  in_=gidx_ext[:, gbase:gbase + 8 * gwtot])
                gqs = []
                colg = 0
                for q in range(NSUB):
                    gwq = int(GW[g, q])
                    if gwq == 0:
                        gqs.append(None)
                        continue
                    gq = gp.tile([128, gwq, TROW], f32, name=f"g2_{g}q{q}",
                                 tag=f"g1q{q}")
                    nidx = 128 * gwq
                    nc.gpsimd.dma_gather(
                        gq[:], tab2_full[WBASES[q]:WBASES[q] + 32768, :],
                        gi[:, 8 * colg:8 * (colg + gwq)],
                        nidx, nidx, TROW, queue_num=q, single_packet=False)
                    gqs.append(gq)
                    colg += gwq
                for j in range(GT):
                    t = GT * g + j
                    wt = int(S[t].sum())
                    gxs = []
                    for q in range(NSUB):
                        sq = int(S[t, q])
                        if sq == 0 or gqs[q] is None:
                            gxs.append((None, 0, 0))
                            continue
                        ofs = int(S[GT * g:t, q].sum())
                        gxs.append((gqs[q], sq, ofs))
                    z = sb.tile([128, wt], f32, name=f"z2_{t}", tag="z0")
                    zs = sb.tile([128, wt], f32, name=f"zs2_{t}", tag="zs0")
                    adc = ad2_all[:, t:t + 1]
                    adb = adc.to_broadcast([128, wt])
                    co = 0
                    for gq, sq, ofs in gxs:
                        if gq is None:
                            continue
                        a_s = gq[:, ofs:ofs + sq, 16:17].rearrange(
                            "p w e -> p (w e)")
                        nc.vector.tensor_tensor(out=z[:, co:co + sq], in0=a_s,
                                                in1=adb[:, co:co + sq],
                                                op=AL.add)
                        co += sq
                    nc.vector.tensor_tensor(
                        out=zs[:], in0=z[:],
                        in1=c02[:, 0:1].to_broadcast([128, wt]), op=AL.mult)
                    nc.vector.tensor_tensor(out=z[:], in0=z[:], in1=zs[:],
                                            op=AL.max)
                    ex = sb.tile([128, wt], f32, name=f"ex2_{t}", tag="ex0")
                    den = sb.tile([128, 1], f32, name=f"den2_{t}", tag="den")
                    nc.scalar.activation(out=ex[:], in_=z[:], func=EXP,
                                         accum_out=den[:, 0:1])
                    msg = sb.tile([128, wt, OUT], f32, name=f"msg2_{t}",
                                  tag="msg")
                    co = 0
                    for gq, sq, ofs in gxs:
                        if gq is None:
                            continue
                        exb = ex[:, co:co + sq, None].to_broadcast(
                            [128, sq, OUT])
                        nc.vector.tensor_tensor(out=msg[:, co:co + sq, :],
                                                in0=gq[:, ofs:ofs + sq, 0:16],
                                                in1=exb, op=AL.mult)
                        co += sq
                    num = sb.tile([128, OUT], f32, name=f"num2_{t}", tag="num")
                    nc.vector.reduce_sum(
                        out=num[:], in_=msg[:].rearrange("p w e -> p e w"),
                        axis=mybir.AxisListType.X)
                    nc.vector.tensor_tensor(out=den[:], in0=den[:],
                                            in1=c30[:, 0:1], op=AL.max)
                    rcp = sb.tile([128, 1], f32, name=f"rcp2_{t}", tag="rcp")
                    nc.vector.reciprocal(out=rcp[:], in_=den[:])
                    o2 = sb.tile([128, OUT], f32, name=f"o2_{t}", tag="o2")
                    nc.vector.tensor_tensor(
                        out=o2[:], in0=num[:],
                        in1=rcp[:, 0:1].to_broadcast([128, OUT]), op=AL.mult)
                    nc.vector.tensor_tensor(out=o2[:], in0=o2[:], in1=b2s[:],
                                            op=AL.add)
                    nc.scalar.activation(out=e_all[:, OUT * t:OUT * (t + 1)],
                                         in_=o2[:], func=EXP)
                    nc.scalar.activation(out=ebf_all[:, OUT * t:OUT * (t + 1)],
                                         in_=e_all[:, OUT * t:OUT * (t + 1)],
                                         func=COPYF)
                    ohf_t = sb.tile([128, WSEG], f8, name=f"ohf{t}", tag="ohf")
                    nc.sync.dma_start(out=ohf_t[:],
                                      in_=ohf_ext[t * 128:(t + 1) * 128, :])
                    for k in range(2):
                        nc.tensor.matmul(out=sp[k][:],
                                         lhsT=ohf_t[:, k * 128:(k + 1) * 128],
                                         rhs=ebf_all[:, OUT * t:OUT * (t + 1)],
                                         start=(t == 0), stop=(t == NT - 1))

            # ---- phase 4: combine segment sums across cores ---------------
            zt = sb.tile([128, 160], f32, name="zt")
            nc.vector.memset(zt[:], 0.0)
            nc.sync.dma_start(
                out=s_loc.rearrange("(c p) f -> p c f", p=128),
                in_=zt[:].rearrange("p (c f) -> p c f", c=10))
            sxi = sb.tile([128, 2], mybir.dt.int32, name="sxi")
            nc.sync.dma_start(out=sxi[:], in_=sidx_ext[:, :])
            for k in range(2):
                spc = sb.tile([128, OUT], f32, name=f"spc{k}", tag="spc")
                nc.vector.tensor_copy(out=spc[:], in_=sp[k][:])
                nc.gpsimd.indirect_dma_start(
                    out=s_loc[:, :],
                    out_offset=IOA(ap=sxi[:, k:k + 1], axis=0),
                    in_=spc[:], in_offset=None)

            nc.gpsimd.collective_compute(
                "AllReduce", AL.add, replica_groups=[list(range(NC))],
                ins=[s_loc.opt()], outs=[s_red.opt()])

            sw = []
            for k in range(2):
                swf = sb.tile([128, OUT], f32, name=f"swf{k}", tag="swf")
                nc.gpsimd.indirect_dma_start(
                    out=swf[:], out_offset=None,
                    in_=s_red[:, :],
                    in_offset=IOA(ap=sxi[:, k:k + 1], axis=0))
                swb = rp.tile([128, OUT], bf16, name=f"sw{k}")
                nc.vector.tensor_copy(out=swb[:], in_=swf[:])
                sw.append(swb)

            # ---- phase 5: divide, write out -------------------------------
            for t in range(NT):
                oht_t = sb.tile([128, WSEG], f8, name=f"oht{t}", tag="oht")
                nc.sync.dma_start(out=oht_t[:],
                                  in_=oht_ext[t * 128:(t + 1) * 128, :])
                dp = pp.tile([128, OUT], f32, name=f"dp{t}", tag="dp")
                for k in range(2):
                    nc.tensor.matmul(out=dp[:],
                                     lhsT=oht_t[:, k * 128:(k + 1) * 128],
                                     rhs=sw[k][:], start=(k == 0), stop=(k == 1))
                dd = sb.tile([128, OUT], f32, name=f"dd{t}", tag="dd")
                nc.vector.tensor_tensor(out=dd[:], in0=dp[:],
                                        in1=c30[:, 0:1].to_broadcast([128, OUT]),
                                        op=AL.max)
                nc.vector.reciprocal(out=dd[:], in_=dd[:])
                nc.vector.tensor_tensor(out=fo_all[:, OUT * t:OUT * (t + 1)],
                                        in0=e_all[:, OUT * t:OUT * (t + 1)],
                                        in1=dd[:], op=AL.mult)
            nc.sync.dma_start(
                out=out_ext.rearrange("(t p) e -> p t e", p=128),
                in_=fo_all[:].rearrange("p (t e) -> p t e", e=OUT))

    nc.compile()
    return nc


def kernel_impl(inputs, trace=False):
    from concourse.bass_utils import run_bass_kernel_spmd
    shared, per_core, asm = _preprocess(**inputs)
    nc = _build(shared)
    res = run_bass_kernel_spmd(nc, per_core, core_ids=list(range(NC)),
                               trace=trace)
    out = np.zeros((N, OUT), dtype=np.float32)
    for c in range(NC):
        o = np.asarray(res.results[c]["out"])
        m = asm["real"][c]
        out[asm["glb"][c][m]] = o[m]
    return out, res


def kernel(**inputs):
    out, _ = kernel_impl(inputs, trace=False)
    return out



# revision 24
# speedup vs baseline: 1.3284x; 1.0778x over previous
"""TRN2 Bass kernel for 2-layer GAT + grouped softmax (nn_Actor_1881195675935).

8-core SPMD. Nodes sharded contiguously (12500/core, padded to 12544 = 98
tiles of 128); edges live with the owner of their dst node in an ELLPACK
layout (partition = dst node, free axis = edge slots, per-core relabeling
minimizes slot padding). Per layer: each core computes [h | a_src] rows for
its nodes (att vectors folded into the weights on host), AllGathers the
table, and fetches per-edge rows with dma_gather (4 SWDGE queues, 256B rows,
int16 indices into 4 subtables). Attention softmax runs on DVE/ACT over the
slot axis (a_dst is a per-partition scalar; slot pads point at a poisoned
row with a_src=-1e4 so exp()==0). The final `index`-grouped softmax uses
baked fp8 one-hot matmuls + a 64KB AllReduce.
"""

import sys

sys.path.insert(0, "/opt/trn_rl_repo")

import numpy as np
import ml_dtypes  # noqa: F401

N = 100000
NPC_REAL = 12500
NPC = 12544               # = 98 * 128
NT = 98
NC = 8
NSUB = 4
SUBROWS = 2 * NPC
WBASES = [0, 22528, 45056, 67584]  # gather window base positions
F_IN = 128
H1, C1 = 2, 16
OUT = 16
WSEG = 256
TROW = 64                 # table row f32 elements (256B)
PAD_AS = -1.0e4
PAD_IDX = 12543
PADPOS = [12543, 37631, 62719, 87807]  # a pad-node position inside each window


def _preprocess(x, edge_index, index, W1, att_src1, att_dst1, b1,
                W2, att_src2, att_dst2, b2):
    f32 = np.float32
    src = np.asarray(edge_index[0], dtype=np.int64)
    dst = np.asarray(edge_index[1], dtype=np.int64)
    loops = np.arange(N, dtype=np.int64)
    src = np.concatenate([src, loops]).astype(np.int64)
    dst = np.concatenate([dst, loops]).astype(np.int64)

    owner_dst = dst // NPC_REAL
    ldst = dst - owner_dst * NPC_REAL

    # window bases: 4 windows of 32768 positions with overlap; edges whose
    # src position falls in an overlap may go to either window.
    WBASE = np.array([0, 22528, 45056, 67584], dtype=np.int64)
    RB = np.array([0, 22528, 32768, 45056, 55296, 67584, 77824, 100352],
                  dtype=np.int64)

    # need positions first: provisional relabeling requires counts; do a
    # two-step: positions depend only on the per-core permutation, which we
    # compute from window profiles, which depend on positions... break the
    # loop: window regions are defined on *positions*, and within-core
    # relabeling permutes positions within one core's 12544-range. Region
    # boundaries (multiples of 22528/32768) do not align with core
    # boundaries (12544), so a node's region can change with relabeling.
    # Use original-order positions for region assignment: pos0(g) =
    # owner*NPC + old_local. Relabeling then permutes *within* the core
    # range; a src's position changes by < NPC which can cross a region
    # boundary. To keep indices exact we compute final positions first with
    # a degree-based permutation, then assign windows from *final*
    # positions.
    counts_deg = np.bincount(owner_dst * NPC + ldst, minlength=NC * NPC)
    counts_deg = counts_deg.reshape(NC, NPC)
    orders = np.zeros((NC, NPC), dtype=np.int64)
    # first pass permutation: by degree (refined below by window profile)
    for c in range(NC):
        orders[c] = np.argsort(-counts_deg[c], kind="stable")
    inv_orders = np.argsort(orders, axis=1)
    pos = np.zeros(N, dtype=np.int64)
    ar = np.arange(NPC_REAL)
    for c in range(NC):
        pos[c * NPC_REAL + ar] = c * NPC + inv_orders[c][ar]

    spos = pos[src]
    region = np.searchsorted(RB, spos, side="right") - 1     # 0..6
    nid = owner_dst * NPC + ldst
    rcnt = np.zeros((NC * NPC, 7), dtype=np.int64)
    np.add.at(rcnt, (nid, region), 1)
    Ccum = np.concatenate([np.zeros((NC * NPC, 1), np.int64),
                           np.cumsum(rcnt, axis=1)], axis=1)  # [n, 8]
    deg = Ccum[:, 7]
    b = np.zeros((NC * NPC, 3), dtype=np.int64)
    for j in range(3):
        tgt = ((j + 1) * deg + 3) // 4
        b[:, j] = np.clip(tgt, Ccum[:, 2 * j + 1], Ccum[:, 2 * j + 2])
    b = np.maximum.accumulate(b, axis=1)
    cuts = np.concatenate([np.zeros((NC * NPC, 1), np.int64), b,
                           deg[:, None]], axis=1)             # [n, 5]
    nW = np.diff(cuts, axis=1)                                # [n, 4]

    # refine relabeling by window profile, then recompute everything that
    # depends on position. Window counts of a node do not depend on its own
    # position (only on its neighbors'), so refining the permutation does
    # change *other* nodes' profiles; accept one iteration (profiles shift
    # by few edges) and recompute regions/cuts after re-permuting.
    def _cluster(prof):
        out = []
        def rec(ids):
            if len(ids) <= 128:
                out.append(ids)
                return
            sub = prof[ids]
            d = int(np.argmax(sub.max(0) - sub.min(0)))
            ids = ids[np.argsort(-sub[:, d], kind="stable")]
            left = (len(ids) // 128 // 2) * 128
            rec(ids[:left]); rec(ids[left:])
        rec(np.arange(len(prof)))
        return np.concatenate(out)

    def _slots_of(order, n):
        nn = n[order].reshape(NT, 128, NSUB)
        return int(nn.max(axis=1).sum())

    for c in range(NC):
        prof = nW[c * NPC:(c + 1) * NPC]
        cand1 = np.lexsort((-prof[:, 3], -prof[:, 2], -prof[:, 1], -prof[:, 0]))
        cand2 = _cluster(prof)
        best = cand1 if _slots_of(cand1, prof) <= _slots_of(cand2, prof) else cand2
        orders[c] = best
    inv_orders = np.argsort(orders, axis=1)
    for c in range(NC):
        pos[c * NPC_REAL + ar] = c * NPC + inv_orders[c][ar]

    spos = pos[src]
    region = np.searchsorted(RB, spos, side="right") - 1
    new_ldst = inv_orders[owner_dst, ldst]
    nid = owner_dst * NPC + new_ldst
    rcnt = np.zeros((NC * NPC, 7), dtype=np.int64)
    np.add.at(rcnt, (nid, region), 1)
    Ccum = np.concatenate([np.zeros((NC * NPC, 1), np.int64),
                           np.cumsum(rcnt, axis=1)], axis=1)
    deg = Ccum[:, 7]
    b = np.zeros((NC * NPC, 3), dtype=np.int64)
    for j in range(3):
        tgt = ((j + 1) * deg + 3) // 4
        b[:, j] = np.clip(tgt, Ccum[:, 2 * j + 1], Ccum[:, 2 * j + 2])
    b = np.maximum.accumulate(b, axis=1)
    cuts = np.concatenate([np.zeros((NC * NPC, 1), np.int64), b,
                           deg[:, None]], axis=1)
    nW = np.diff(cuts, axis=1)                                # [n, 4]

    # minimax cut refinement: shave per-(tile,window) maxima by moving
    # single edges across window cuts where the overlap regions allow.
    LO = np.stack([Ccum[:, 2 * j + 1] for j in range(3)], 1)
    HI = np.stack([Ccum[:, 2 * j + 2] for j in range(3)], 1)
    tile_of = np.tile(np.repeat(np.arange(NT), 128), NC)
    for _ in range(40):
        nW = np.diff(cuts, axis=1)
        Sit = nW.reshape(NC, NT, 128, NSUB).max(axis=(0, 2))
        Sn = Sit[tile_of]
        moved = 0
        for j in range(1, 4):
            can = ((nW[:, j - 1] == Sn[:, j - 1]) & (nW[:, j] + 1 < Sn[:, j])
                   & (cuts[:, j] - 1 >= LO[:, j - 1])
                   & (cuts[:, j] - 1 >= cuts[:, j - 1]))
            cuts[:, j] -= can
            can2 = ((nW[:, j] == Sn[:, j]) & (nW[:, j - 1] + 1 < Sn[:, j - 1])
                    & (cuts[:, j] + 1 <= HI[:, j - 1])
                    & (cuts[:, j] + 1 <= cuts[:, j + 1]) & ~can)
            cuts[:, j] += can2
            moved += int(can.sum()) + int(can2.sum())
            nW = np.diff(cuts, axis=1)
        if moved == 0:
            break
    nW = np.diff(cuts, axis=1)

    ncounts = nW.reshape(NC, NPC, NSUB)
    S = ncounts.reshape(NC, NT, 128, NSUB).max(axis=(0, 2))   # [NT, NSUB]

    # edges sorted by (node, src position)
    eorder = np.lexsort((spos, nid))
    s_spos = spos[eorder]
    run_starts = np.zeros(NC * NPC + 1, dtype=np.int64)
    np.cumsum(np.bincount(nid, minlength=NC * NPC), out=run_starts[1:])

    # group pairs of tiles into one gather per window: fewer, larger SWDGE
    # desc-gen instructions (the gen rate is the kernel's floor).
    GT = 1
    NG = NT // GT
    GW = np.zeros((NG, NSUB), dtype=np.int64)   # group window widths
    for g in range(NG):
        GW[g] = S[GT * g:GT * (g + 1)].sum(axis=0)
    gwt = GW.sum(axis=1)
    gidx_off = np.concatenate([[0], np.cumsum(8 * gwt)]).astype(np.int64)
    IDXW = int(gidx_off[-1])

    def _block(c, t, w):
        sq = int(S[t, w])
        nodes = c * NPC + t * 128 + np.arange(128)
        r0n = run_starts[nodes]
        lo = cuts[nodes, w]
        nq = nW[nodes, w]
        i = np.arange(sq)[:, None]
        mask = i < nq[None, :]
        gi_ = np.minimum(r0n[None, :] + lo[None, :] + i, len(s_spos) - 1)
        padw = PADPOS[w] - int(WBASE[w])
        return np.where(mask, s_spos[gi_] - int(WBASE[w]), padw)  # [sq, 128]

    gidx = np.zeros((NC, 128, IDXW), dtype=np.int16)
    for c in range(NC):
        for g in range(NG):
            parts = []
            for w in range(NSUB):
                for t in range(GT * g, GT * (g + 1)):
                    if int(S[t, w]):
                        parts.append(_block(c, t, w))
            flat = np.concatenate(parts, axis=0)       # [gwt[g], 128]
            w16 = flat.reshape(-1, 16).T.astype(np.int16)
            gidx[c, :, gidx_off[g]:gidx_off[g + 1]] = np.tile(w16, (8, 1))

    W1 = np.asarray(W1, f32); W2 = np.asarray(W2, f32)
    as1 = np.asarray(att_src1, f32); ad1 = np.asarray(att_dst1, f32)
    as2 = np.asarray(att_src2, f32); ad2 = np.asarray(att_dst2, f32)
    vs1 = np.stack([W1[:, h * C1:(h + 1) * C1] @ as1[h] for h in range(H1)], 1)
    vd1 = np.stack([W1[:, h * C1:(h + 1) * C1] @ ad1[h] for h in range(H1)], 1)
    wcat1 = np.concatenate([W1, vs1, vd1], axis=1).astype(f32)
    vs2 = (W2 @ as2[0])[:, None]
    vd2 = (W2 @ ad2[0])[:, None]
    wcat2 = np.concatenate([W2, vs2, vd2], axis=1).astype(f32)

    x = np.asarray(x, f32)
    xT = np.zeros((NC, F_IN, NPC), dtype=f32)
    glb = np.zeros((NC, NPC), dtype=np.int64)
    real = np.zeros((NC, NPC), dtype=bool)
    for c in range(NC):
        ol = orders[c]
        is_real = ol < NPC_REAL
        g = np.where(is_real, c * NPC_REAL + np.minimum(ol, NPC_REAL - 1), 0)
        xT[c] = np.where(is_real[:, None], x[g], 0.0).astype(f32).T
        glb[c] = g
        real[c] = is_real

    index = np.asarray(index, np.int64)
    seg = np.zeros((NC, NPC), dtype=np.int64)
    g0 = np.zeros(NC, dtype=np.int64)
    for c in range(NC):
        seg[c] = np.where(real[c], index[glb[c]], 0)
        s = seg[c][real[c]]
        g0[c] = s.min()
        assert s.max() - s.min() < WSEG, "segment window exceeds WSEG"
    f8 = ml_dtypes.float8_e4m3
    # ohf[c]: [NT*128, 256]  (lhsT chunks along free); oht[c]: [NT*128, 256]
    ohf = np.zeros((NC, NT * 128, WSEG), dtype=f8)
    oht = np.zeros((NC, NT * 128, WSEG), dtype=f8)
    for c in range(NC):
        for t in range(NT):
            sl = seg[c, t * 128:(t + 1) * 128] - g0[c]
            m = real[c, t * 128:(t + 1) * 128]
            oh = np.zeros((128, WSEG), dtype=np.float32)
            oh[np.arange(128)[m], sl[m]] = 1.0
            ohf[c, t * 128:(t + 1) * 128] = oh.astype(f8)
            # bwd lhsT chunk k: [128 segs, 128 nodes] -> store as [128, 2*128]
            ohtk = np.concatenate([oh[:, :128].T, oh[:, 128:].T], axis=1)
            oht[c, t * 128:(t + 1) * 128] = ohtk.astype(f8)

    padfix = np.zeros((128, 3), dtype=f32)
    padfix[84:128, :] = PAD_AS

    sidx = np.zeros((NC, 128, 2), dtype=np.int32)
    for c in range(NC):
        for k in range(2):
            sidx[c, :, k] = g0[c] + k * 128 + np.arange(128)

    b1t = np.tile(np.asarray(b1, f32)[None, :], (128, 1)).astype(f32)
    b2t = np.tile(np.asarray(b2, f32)[None, :], (128, 1)).astype(f32)

    per_core = [{
        "xT": np.ascontiguousarray(xT[c]),
        "wcat1": wcat1, "wcat2": wcat2, "b1t": b1t, "b2t": b2t,
        "gidx": np.ascontiguousarray(gidx[c]),
        "padfix": padfix,
        "ohf": np.ascontiguousarray(ohf[c]),
        "oht": np.ascontiguousarray(oht[c]),
        "sidx": np.ascontiguousarray(sidx[c]),
    } for c in range(NC)]
    shared = {"S": S, "GT": GT, "NG": NG, "GW": GW, "gidx_off": gidx_off,
              "IDXW": IDXW}
    asm = {"glb": glb, "real": real}
    return shared, per_core, asm


def _build(shared):
    import concourse.bass as bass
    import concourse.bacc as bacc
    import concourse.tile as tile
    from concourse import mybir, library_config
    from concourse.masks import make_identity

    S = shared["S"]; IDXW = shared["IDXW"]
    GT = shared["GT"]; NG = shared["NG"]; GW = shared["GW"]
    gidx_off = shared["gidx_off"]
    f32 = mybir.dt.float32
    bf16 = mybir.dt.bfloat16
    f8 = mybir.dt.float8e4
    i16 = mybir.dt.int16
    AL = mybir.AluOpType
    EXP = mybir.ActivationFunctionType.Exp
    COPYF = mybir.ActivationFunctionType.Copy
    RELU = mybir.ActivationFunctionType.Relu
    IOA = bass.IndirectOffsetOnAxis

    nc = bacc.Bacc("TRN2", target_bir_lowering=False, debug=False,
                   num_devices=NC, num_swdge_queues=4)

    xT_ext = nc.dram_tensor("xT", [F_IN, NPC], f32, kind="ExternalInput")
    wcat1_ext = nc.dram_tensor("wcat1", [F_IN, 36], f32, kind="ExternalInput")
    wcat2_ext = nc.dram_tensor("wcat2", [32, 18], f32, kind="ExternalInput")
    b1_ext = nc.dram_tensor("b1t", [128, 32], f32, kind="ExternalInput")
    b2_ext = nc.dram_tensor("b2t", [128, 16], f32, kind="ExternalInput")
    gidx_ext = nc.dram_tensor("gidx", [128, IDXW], i16, kind="ExternalInput")
    ohf_ext = nc.dram_tensor("ohf", [NT * 128, WSEG], f8, kind="ExternalInput")
    oht_ext = nc.dram_tensor("oht", [NT * 128, WSEG], f8, kind="ExternalInput")
    sidx_ext = nc.dram_tensor("sidx", [128, 2], mybir.dt.int32, kind="ExternalInput")
    padfix_ext = nc.dram_tensor("padfix", [128, 3], f32, kind="ExternalInput")
    out_ext = nc.dram_tensor("out", [NPC, OUT], f32, kind="ExternalOutput")

    with tile.TileContext(nc) as tc:
        with (
            tc.tile_pool(name="dram", bufs=1, space="DRAM") as dr,
            tc.tile_pool(name="const", bufs=1) as cpool,
            tc.tile_pool(name="sbuf", bufs=4) as sb,
            tc.tile_pool(name="gat", bufs=4) as gp,
            tc.tile_pool(name="gia", bufs=6) as gia,
            tc.tile_pool(name="psum", bufs=2, space="PSUM") as pp,
            tc.tile_pool(name="psum2", bufs=1, space="PSUM") as pp2,
            tc.tile_pool(name="psum_seg", bufs=1, space="PSUM") as pseg,
            tc.tile_pool(name="res", bufs=1) as rp,
        ):
            tab1_loc = dr.tile([NPC, TROW], f32, name="tab1_loc")
            tab2_loc = dr.tile([NPC, TROW], f32, name="tab2_loc")
            tab1_full = dr.tile([NC * NPC, TROW], f32, name="tab1_full",
                                addr_space="Shared")
            tab2_full = dr.tile([NC * NPC, TROW], f32, name="tab2_full",
                                addr_space="Shared")
            s_loc = dr.tile([1280, OUT], f32, name="s_loc")
            s_red = dr.tile([1280, OUT], f32, name="s_red", addr_space="Shared")

            nc.gpsimd.load_library(library_config.mlp)

            ident = cpool.tile([128, 128], f32, name="ident")
            make_identity(nc, ident[:])
            wc1 = cpool.tile([F_IN, 36], f32, name="wc1")
            nc.sync.dma_start(out=wc1[:], in_=wcat1_ext[:, :])
            wc2 = cpool.tile([32, 18], f32, name="wc2")
            nc.sync.dma_start(out=wc2[:], in_=wcat2_ext[:, :])
            b1s = cpool.tile([128, 32], f32, name="b1s")
            nc.sync.dma_start(out=b1s[:], in_=b1_ext[:, :])
            b2s = cpool.tile([128, 16], f32, name="b2s")
            nc.sync.dma_start(out=b2s[:], in_=b2_ext[:, :])
            pfx = cpool.tile([128, 3], f32, name="pfx")
            nc.sync.dma_start(out=pfx[:], in_=padfix_ext[:, :])

            ad1_all = rp.tile([128, NT * 2], f32, name="ad1_all")
            ad2_all = rp.tile([128, NT], f32, name="ad2_all")
            e_all = rp.tile([128, NT * OUT], f32, name="e_all")
            ebf_all = rp.tile([128, NT * OUT], bf16, name="ebf_all")
            x2_all = rp.tile([128, NT * 32], f32, name="x2_all")
            hs_all = rp.tile([128, NT * 36], f32, name="hs_all")
            h2s_all = rp.tile([128, NT * 18], f32, name="h2s_all")
            fo_all = rp.tile([128, NT * OUT], f32, name="fo_all")

            c02 = cpool.tile([128, 1], f32, name="c02")
            nc.vector.memset(c02[:], 0.2)
            c30 = cpool.tile([128, 1], f32, name="c30")
            nc.vector.memset(c30[:], 1e-30)

            # ---- phase 0: table1 rows -------------------------------------
            for t in range(NT):
                xt = sb.tile([128, 128], f32, name=f"xt{t}", tag="xt")
                nc.sync.dma_start(out=xt[:], in_=xT_ext[:, t * 128:(t + 1) * 128])
                hp = pp.tile([128, 36], f32, name=f"hp{t}", tag="hp")
                nc.tensor.matmul(out=hp[:], lhsT=xt[:], rhs=wc1[:],
                                 start=True, stop=True)
                hs = hs_all[:, 36 * t:36 * (t + 1)]
                nc.vector.tensor_copy(out=hs, in_=hp[:])
                if t == NT - 1:
                    nc.vector.tensor_tensor(out=hs_all[:, 36 * t + 32:36 * t + 34],
                                            in0=hs_all[:, 36 * t + 32:36 * t + 34],
                                            in1=pfx[:, 0:2], op=AL.add)
                nc.vector.tensor_copy(out=ad1_all[:, 2 * t:2 * t + 2],
                                      in_=hs_all[:, 36 * t + 34:36 * t + 36])
            nc.sync.dma_start(
                out=tab1_loc.rearrange("(t p) e -> p t e", p=128)[:, :, 0:36],
                in_=hs_all[:].rearrange("p (t e) -> p t e", e=36))

            nc.gpsimd.collective_compute(
                "AllGather", AL.bypass, replica_groups=[list(range(NC))],
                ins=[tab1_loc.opt()], outs=[tab1_full.opt()])

            # ---- phase 1 (+fused table2 rows), grouped gathers ------------
            for g in range(NG):
                gbase = int(gidx_off[g])
                gwtot = int(GW[g].sum())
                gi = gia.tile([128, 8 * gwtot], i16, name=f"gi1_{g}", tag="gi")
                nc.sync.dma_start(out=gi[:],
                                  in_=gidx_ext[:, gbase:gbase + 8 * gwtot])
                gqs = []
                colg = 0
                for q in range(NSUB):
                    gwq = int(GW[g, q])
                    if gwq == 0:
                        gqs.append(None)
                        continue
                    gq = gp.tile([128, gwq, TROW], f32, name=f"g1_{g}q{q}",
                                 tag=f"g1q{q}")
                    nidx = 128 * gwq
                    nc.gpsimd.dma_gather(
                        gq[:], tab1_full[WBASES[q]:WBASES[q] + 32768, :],
                        gi[:, 8 * colg:8 * (colg + gwq)],
                        nidx, nidx, TROW, queue_num=q, single_packet=False)
                    gqs.append(gq)
                    colg += gwq
                for j in range(GT):
                    t = GT * g + j
                    wt = int(S[t].sum())
                    gxs = []
                    for q in range(NSUB):
                        sq = int(S[t, q])
                        if sq == 0 or gqs[q] is None:
                            gxs.append((None, 0, 0))
                            continue
                        ofs = int(S[GT * g:t, q].sum())
                        gxs.append((gqs[q], sq, ofs))
                    x2 = sb.tile([128, 32], f32, name=f"x2_{t}", tag="x2")
                    for h in range(H1):
                        z = sb.tile([128, wt], f32, name=f"z{t}h{h}", tag=f"z{h}")
                        zs = sb.tile([128, wt], f32, name=f"zs{t}h{h}",
                                     tag=f"zs{h}")
                        adc = ad1_all[:, 2 * t + h:2 * t + h + 1]
                        adb = adc.to_broadcast([128, wt])
                        co = 0
                        for gq, sq, ofs in gxs:
                            if gq is None:
                                continue
                            a_s = gq[:, ofs:ofs + sq, 32 + h:33 + h].rearrange(
                                "p w e -> p (w e)")
                            nc.vector.tensor_tensor(out=z[:, co:co + sq],
                                                    in0=a_s,
                                                    in1=adb[:, co:co + sq],
                                                    op=AL.add)
                            co += sq
                        nc.vector.tensor_tensor(
                            out=zs[:], in0=z[:],
                            in1=c02[:, 0:1].to_broadcast([128, wt]), op=AL.mult)
                        nc.vector.tensor_tensor(out=z[:], in0=z[:], in1=zs[:],
                                                op=AL.max)
                        ex = sb.tile([128, wt], f32, name=f"ex{t}h{h}",
                                     tag=f"ex{h}")
                        den = sb.tile([128, 1], f32, name=f"den{t}h{h}",
                                      tag="den")
                        nc.scalar.activation(out=ex[:], in_=z[:], func=EXP,
                                             accum_out=den[:, 0:1])
                        msg = sb.tile([128, wt, C1], f32, name=f"msg{t}h{h}",
                                      tag="msg")
                        co = 0
                        for gq, sq, ofs in gxs:
                            if gq is None:
                                continue
                            exb = ex[:, co:co + sq, None].to_broadcast(
                                [128, sq, C1])
                            nc.vector.tensor_tensor(
                                out=msg[:, co:co + sq, :],
                                in0=gq[:, ofs:ofs + sq, 16 * h:16 * h + 16],
                                in1=exb, op=AL.mult)
                            co += sq
                        num = sb.tile([128, C1], f32, name=f"num{t}h{h}",
                                      tag="num")
                        nc.vector.reduce_sum(
                            out=num[:], in_=msg[:].rearrange("p w e -> p e w"),
                            axis=mybir.AxisListType.X)
                        nc.vector.tensor_tensor(out=den[:], in0=den[:],
                                                in1=c30[:, 0:1], op=AL.max)
                        rcp = sb.tile([128, 1], f32, name=f"rcp{t}h{h}",
                                      tag="rcp")
                        nc.vector.reciprocal(out=rcp[:], in_=den[:])
                        nc.vector.tensor_tensor(
                            out=x2[:, 16 * h:16 * h + 16], in0=num[:],
                            in1=rcp[:, 0:1].to_broadcast([128, C1]), op=AL.mult)
                    nc.vector.tensor_tensor(out=x2[:], in0=x2[:], in1=b1s[:],
                                            op=AL.add)
                    nc.scalar.activation(out=x2_all[:, 32 * t:32 * (t + 1)],
                                         in_=x2[:], func=RELU)
                    # fused table2 row for tile t
                    x2tp = pp2.tile([32, 128], f32, name=f"x2tp{t}", tag="x2tp")
                    nc.tensor.transpose(out=x2tp[:],
                                        in_=x2_all[:, 32 * t:32 * (t + 1)],
                                        identity=ident[:])
                    x2ts = sb.tile([32, 128], f32, name=f"x2ts{t}", tag="x2ts")
                    nc.scalar.activation(out=x2ts[:], in_=x2tp[:], func=COPYF)
                    h2p = pp2.tile([128, 18], f32, name=f"h2p{t}", tag="h2p")
                    nc.tensor.matmul(out=h2p[:], lhsT=x2ts[:], rhs=wc2[:],
                                     start=True, stop=True)
                    h2s = h2s_all[:, 18 * t:18 * (t + 1)]
                    nc.scalar.activation(out=h2s, in_=h2p[:], func=COPYF)
                    if t == NT - 1:
                        nc.vector.tensor_tensor(
                            out=h2s_all[:, 18 * t + 16:18 * t + 17],
                            in0=h2s_all[:, 18 * t + 16:18 * t + 17],
                            in1=pfx[:, 2:3], op=AL.add)
                    nc.scalar.activation(out=ad2_all[:, t:t + 1],
                                         in_=h2s_all[:, 18 * t + 17:18 * t + 18],
                                         func=COPYF)
            nc.sync.dma_start(
                out=tab2_loc.rearrange("(t p) e -> p t e", p=128)[:, :, 0:18],
                in_=h2s_all[:].rearrange("p (t e) -> p t e", e=18))

            nc.gpsimd.collective_compute(
                "AllGather", AL.bypass, replica_groups=[list(range(NC))],
                ins=[tab2_loc.opt()], outs=[tab2_full.opt()])

            # ---- phase 3: layer-2 aggregation + exp + segment partials ----
            sp = [pseg.tile([128, OUT], f32, name=f"segp{k}") for k in range(2)]
            for g in range(NG):
                gbase = int(gidx_off[g])
                gwtot = int(GW[g].sum())
                gi = gia.tile([128, 8 * gwtot], i16, name=f"gi2_{g}", tag="gi")
                nc.sync.dma_start(out=gi[:],
                                  in_=gidx_ext[:, gbase:gbase + 8 * gwtot])
                gqs = []
                colg = 0
                for q in range(NSUB):
                    gwq = int(GW[g, q])
                    if gwq == 0:
                        gqs.append(None)
                        continue
                    gq = gp.tile([128, gwq, TROW], f32, name=f"g2_{g}q{q}",
                                 tag=f"g1q{q}")
                    nidx = 128 * gwq
                    nc.gpsimd.dma_gather(
                        gq[:], tab2_full[WBASES[q]:WBASES[q] + 32768, :],
                        gi[:, 8 * colg:8 * (colg + gwq)],
                        nidx, nidx, TROW, queue_num=q, single_packet=False)
                    gqs.append(gq)
                    colg += gwq
                for j in range(GT):
                    t = GT * g + j
                    wt = int(S[t].sum())
                    gxs = []
                    for q in range(NSUB):
                        sq = int(S[t, q])
                        if sq == 0 or gqs[q] is None:
                            gxs.append((None, 0, 0))
                            continue
                        ofs = int(S[GT * g:t, q].sum())
                        gxs.append((gqs[q], sq, ofs))
                    z = sb.tile([128, wt], f32, name=f"z2_{t}", tag="z0")
                    zs = sb.tile([128, wt], f32, name=f"zs2_{t}", tag="zs0")
                    adc = ad2_all[:, t:t + 1]
                    adb = adc.to_broadcast([128, wt])
                    co = 0
                    for gq, sq, ofs in gxs:
                        if gq is None:
                            continue
                        a_s = gq[:, ofs:ofs + sq, 16:17].rearrange(
                            "p w e -> p (w e)")
                        nc.vector.tensor_tensor(out=z[:, co:co + sq], in0=a_s,
                                                in1=adb[:, co:co + sq],
                                                op=AL.add)
                        co += sq
                    nc.vector.tensor_tensor(
                        out=zs[:], in0=z[:],
                        in1=c02[:, 0:1].to_broadcast([128, wt]), op=AL.mult)
                    nc.vector.tensor_tensor(out=z[:], in0=z[:], in1=zs[:],
                                            op=AL.max)
                    ex = sb.tile([128, wt], f32, name=f"ex2_{t}", tag="ex0")
                    den = sb.tile([128, 1], f32, name=f"den2_{t}", tag="den")
                    nc.scalar.activation(out=ex[:], in_=z[:], func=EXP,
                                         accum_out=den[:, 0:1])
                    msg = sb.tile([128, wt, OUT], f32, name=f"msg2_{t}",
                                  tag="msg")
                    co = 0
                    for gq, sq, ofs in gxs:
                        if gq is None:
                            continue
                        exb = ex[:, co:co + sq, None].to_broadcast(
                            [128, sq, OUT])
                        nc.vector.tensor_tensor(out=msg[:, co:co + sq, :],
                                                in0=gq[:, ofs:ofs + sq, 0:16],
                                                in1=exb, op=AL.mult)
                        co += sq
                    num = sb.tile([128, OUT], f32, name=f"num2_{t}", tag="num")
                    nc.vector.reduce_sum(
                        out=num[:], in_=msg[:].rearrange("p w e -> p e w"),
                        axis=mybir.AxisListType.X)
                    nc.vector.tensor_tensor(out=den[:], in0=den[:],
                                            in1=c30[:, 0:1], op=AL.max)
                    rcp = sb.tile([128, 1], f32, name=f"rcp2_{t}", tag="rcp")
                    nc.vector.reciprocal(out=rcp[:], in_=den[:])
                    o2 = sb.tile([128, OUT], f32, name=f"o2_{t}", tag="o2")
                    nc.vector.tensor_tensor(
                        out=o2[:], in0=num[:],
                        in1=rcp[:, 0:1].to_broadcast([128, OUT]), op=AL.mult)
                    nc.vector.tensor_tensor(out=o2[:], in0=o2[:], in1=b2s[:],
                                            op=AL.add)
                    nc.scalar.activation(out=e_all[:, OUT * t:OUT * (t + 1)],
                                         in_=o2[:], func=EXP)
                    nc.scalar.activation(out=ebf_all[:, OUT * t:OUT * (t + 1)],
                                         in_=e_all[:, OUT * t:OUT * (t + 1)],
                                         func=COPYF)
                    ohf_t = sb.tile([128, WSEG], f8, name=f"ohf{t}", tag="ohf")
                    nc.sync.dma_start(out=ohf_t[:],
                                      in_=ohf_ext[t * 128:(t + 1) * 128, :])
                    for k in range(2):
                        nc.tensor.matmul(out=sp[k][:],
                                         lhsT=ohf_t[:, k * 128:(k + 1) * 128],
                                         rhs=ebf_all[:, OUT * t:OUT * (t + 1)],
                                         start=(t == 0), stop=(t == NT - 1))

            # ---- phase 4: combine segment sums across cores ---------------
            zt = sb.tile([128, 160], f32, name="zt")
            nc.vector.memset(zt[:], 0.0)
            nc.sync.dma_start(
                out=s_loc.rearrange("(c p) f -> p c f", p=128),
                in_=zt[:].rearrange("p (c f) -> p c f", c=10))
            sxi = sb.tile([128, 2], mybir.dt.int32, name="sxi")
            nc.sync.dma_start(out=sxi[:], in_=sidx_ext[:, :])
            for k in range(2):
                spc = sb.tile([128, OUT], f32, name=f"spc{k}", tag="spc")
                nc.vector.tensor_copy(out=spc[:], in_=sp[k][:])
                nc.gpsimd.indirect_dma_start(
                    out=s_loc[:, :],
                    out_offset=IOA(ap=sxi[:, k:k + 1], axis=0),
                    in_=spc[:], in_offset=None)

            nc.gpsimd.collective_compute(
                "AllReduce", AL.add, replica_groups=[list(range(NC))],
                ins=[s_loc.opt()], outs=[s_red.opt()])

            sw = []
            for k in range(2):
                swf = sb.tile([128, OUT], f32, name=f"swf{k}", tag="swf")
                nc.gpsimd.indirect_dma_start(
                    out=swf[:], out_offset=None,
                    in_=s_red[:, :],
                    in_offset=IOA(ap=sxi[:, k:k + 1], axis=0))
                swb = rp.tile([128, OUT], bf16, name=f"sw{k}")
                nc.vector.tensor_copy(out=swb[:], in_=swf[:])
                sw.append(swb)

            # ---- phase 5: divide, write out -------------------------------
            for t in range(NT):
                oht_t = sb.tile([128, WSEG], f8, name=f"oht{t}", tag="oht")
                nc.sync.dma_start(out=oht_t[:],
                                  in_=oht_ext[t * 128:(t + 1) * 128, :])
                dp = pp.tile([128, OUT], f32, name=f"dp{t}", tag="dp")
                for k in range(2):
                    nc.tensor.matmul(out=dp[:],
                                     lhsT=oht_t[:, k * 128:(k + 1) * 128],
                                     rhs=sw[k][:], start=(k == 0), stop=(k == 1))
                dd = sb.tile([128, OUT], f32, name=f"dd{t}", tag="dd")
                nc.vector.tensor_tensor(out=dd[:], in0=dp[:],
                                        in1=c30[:, 0:1].to_broadcast([128, OUT]),
                                        op=AL.max)
                nc.vector.reciprocal(out=dd[:], in_=dd[:])
                nc.vector.tensor_tensor(out=fo_all[:, OUT * t:OUT * (t + 1)],
                                        in0=e_all[:, OUT * t:OUT * (t + 1)],
                                        in1=dd[:], op=AL.mult)
            nc.sync.dma_start(
                out=out_ext.rearrange("(t p) e -> p t e", p=128),
                in_=fo_all[:].rearrange("p (t e) -> p t e", e=OUT))

    nc.compile()
    return nc


def kernel_impl(inputs, trace=False):
    from concourse.bass_utils import run_bass_kernel_spmd
    shared, per_core, asm = _preprocess(**inputs)
    nc = _build(shared)
    res = run_bass_kernel_spmd(nc, per_core, core_ids=list(range(NC)),
                               trace=trace)
    out = np.zeros((N, OUT), dtype=np.float32)
    for c in range(NC):
        o = np.asarray(res.results[c]["out"])
        m = asm["real"][c]
        out[asm["glb"][c][m]] = o[m]
    return out, res


def kernel(**inputs):
    out, _ = kernel_impl(inputs, trace=False)
    return out



# revision 26
# speedup vs baseline: 1.4923x; 1.1234x over previous
"""TRN2 Bass kernel for 2-layer GAT + grouped softmax (nn_Actor_1881195675935).

8-core SPMD. Nodes sharded contiguously (12500/core, padded to 12544 = 98
tiles of 128); edges live with the owner of their dst node in an ELLPACK
layout (partition = dst node, free axis = edge slots, per-core relabeling
minimizes slot padding). Per layer: each core computes [h | a_src] rows for
its nodes (att vectors folded into the weights on host), AllGathers the
table, and fetches per-edge rows with dma_gather (4 SWDGE queues, 256B rows,
int16 indices into 4 subtables). Attention softmax runs on DVE/ACT over the
slot axis (a_dst is a per-partition scalar; slot pads point at a poisoned
row with a_src=-1e4 so exp()==0). The final `index`-grouped softmax uses
baked fp8 one-hot matmuls + a 64KB AllReduce.
"""

import sys

sys.path.insert(0, "/opt/trn_rl_repo")

import numpy as np
import ml_dtypes  # noqa: F401

N = 100000
NPC_REAL = 12500
NPC = 12544               # = 98 * 128
NT = 98
NC = 8
NSUB = 4
SUBROWS = 2 * NPC
WBASES = [0, 22528, 45056, 67584]  # gather window base positions
F_IN = 128
H1, C1 = 2, 16
OUT = 16
WSEG = 256
TROW = 64                 # table row f32 elements (256B)
PAD_AS = -1.0e4
PAD_IDX = 12543
PADPOS = [12543, 37631, 62719, 87807]  # a pad-node position inside each window


def _preprocess(x, edge_index, index, W1, att_src1, att_dst1, b1,
                W2, att_src2, att_dst2, b2):
    f32 = np.float32
    src = np.asarray(edge_index[0], dtype=np.int64)
    dst = np.asarray(edge_index[1], dtype=np.int64)
    loops = np.arange(N, dtype=np.int64)
    src = np.concatenate([src, loops]).astype(np.int64)
    dst = np.concatenate([dst, loops]).astype(np.int64)

    owner_dst = dst // NPC_REAL
    ldst = dst - owner_dst * NPC_REAL

    # window bases: 4 windows of 32768 positions with overlap; edges whose
    # src position falls in an overlap may go to either window.
    WBASE = np.array([0, 22528, 45056, 67584], dtype=np.int64)
    RB = np.array([0, 22528, 32768, 45056, 55296, 67584, 77824, 100352],
                  dtype=np.int64)

    # need positions first: provisional relabeling requires counts; do a
    # two-step: positions depend only on the per-core permutation, which we
    # compute from window profiles, which depend on positions... break the
    # loop: window regions are defined on *positions*, and within-core
    # relabeling permutes positions within one core's 12544-range. Region
    # boundaries (multiples of 22528/32768) do not align with core
    # boundaries (12544), so a node's region can change with relabeling.
    # Use original-order positions for region assignment: pos0(g) =
    # owner*NPC + old_local. Relabeling then permutes *within* the core
    # range; a src's position changes by < NPC which can cross a region
    # boundary. To keep indices exact we compute final positions first with
    # a degree-based permutation, then assign windows from *final*
    # positions.
    counts_deg = np.bincount(owner_dst * NPC + ldst, minlength=NC * NPC)
    counts_deg = counts_deg.reshape(NC, NPC)
    orders = np.zeros((NC, NPC), dtype=np.int64)
    # first pass permutation: by degree (refined below by window profile)
    for c in range(NC):
        orders[c] = np.argsort(-counts_deg[c], kind="stable")
    inv_orders = np.argsort(orders, axis=1)
    pos = np.zeros(N, dtype=np.int64)
    ar = np.arange(NPC_REAL)
    for c in range(NC):
        pos[c * NPC_REAL + ar] = c * NPC + inv_orders[c][ar]

    spos = pos[src]
    region = np.searchsorted(RB, spos, side="right") - 1     # 0..6
    nid = owner_dst * NPC + ldst
    rcnt = np.zeros((NC * NPC, 7), dtype=np.int64)
    np.add.at(rcnt, (nid, region), 1)
    Ccum = np.concatenate([np.zeros((NC * NPC, 1), np.int64),
                           np.cumsum(rcnt, axis=1)], axis=1)  # [n, 8]
    deg = Ccum[:, 7]
    b = np.zeros((NC * NPC, 3), dtype=np.int64)
    for j in range(3):
        tgt = ((j + 1) * deg + 3) // 4
        b[:, j] = np.clip(tgt, Ccum[:, 2 * j + 1], Ccum[:, 2 * j + 2])
    b = np.maximum.accumulate(b, axis=1)
    cuts = np.concatenate([np.zeros((NC * NPC, 1), np.int64), b,
                           deg[:, None]], axis=1)             # [n, 5]
    nW = np.diff(cuts, axis=1)                                # [n, 4]

    # refine relabeling by window profile, then recompute everything that
    # depends on position. Window counts of a node do not depend on its own
    # position (only on its neighbors'), so refining the permutation does
    # change *other* nodes' profiles; accept one iteration (profiles shift
    # by few edges) and recompute regions/cuts after re-permuting.
    def _cluster(prof):
        out = []
        def rec(ids):
            if len(ids) <= 128:
                out.append(ids)
                return
            sub = prof[ids]
            d = int(np.argmax(sub.max(0) - sub.min(0)))
            ids = ids[np.argsort(-sub[:, d], kind="stable")]
            left = (len(ids) // 128 // 2) * 128
            rec(ids[:left]); rec(ids[left:])
        rec(np.arange(len(prof)))
        return np.concatenate(out)

    def _slots_of(order, n):
        nn = n[order].reshape(NT, 128, NSUB)
        return int(nn.max(axis=1).sum())

    for c in range(NC):
        prof = nW[c * NPC:(c + 1) * NPC]
        cand1 = np.lexsort((-prof[:, 3], -prof[:, 2], -prof[:, 1], -prof[:, 0]))
        cand2 = _cluster(prof)
        best = cand1 if _slots_of(cand1, prof) <= _slots_of(cand2, prof) else cand2
        orders[c] = best
    inv_orders = np.argsort(orders, axis=1)
    tile_of = np.tile(np.repeat(np.arange(NT), 128), NC)
    for round_ in range(3):
        for c in range(NC):
            pos[c * NPC_REAL + ar] = c * NPC + inv_orders[c][ar]
        spos = pos[src]
        region = np.searchsorted(RB, spos, side="right") - 1
        new_ldst = inv_orders[owner_dst, ldst]
        nid = owner_dst * NPC + new_ldst
        rcnt = np.zeros((NC * NPC, 7), dtype=np.int64)
        np.add.at(rcnt, (nid, region), 1)
        Ccum = np.concatenate([np.zeros((NC * NPC, 1), np.int64),
                               np.cumsum(rcnt, axis=1)], axis=1)
        deg = Ccum[:, 7]
        b = np.zeros((NC * NPC, 3), dtype=np.int64)
        for j in range(3):
            tgt = ((j + 1) * deg + 3) // 4
            b[:, j] = np.clip(tgt, Ccum[:, 2 * j + 1], Ccum[:, 2 * j + 2])
        b = np.maximum.accumulate(b, axis=1)
        cuts = np.concatenate([np.zeros((NC * NPC, 1), np.int64), b,
                               deg[:, None]], axis=1)
        # minimax cut refinement: shave per-(tile,window) maxima by moving
        # single edges across window cuts where the overlap regions allow.
        LO = np.stack([Ccum[:, 2 * j + 1] for j in range(3)], 1)
        HI = np.stack([Ccum[:, 2 * j + 2] for j in range(3)], 1)
        for _ in range(40):
            nW = np.diff(cuts, axis=1)
            Sit = nW.reshape(NC, NT, 128, NSUB).max(axis=(0, 2))
            Sn = Sit[tile_of]
            moved = 0
            for j in range(1, 4):
                can = ((nW[:, j - 1] == Sn[:, j - 1])
                       & (nW[:, j] + 1 < Sn[:, j])
                       & (cuts[:, j] - 1 >= LO[:, j - 1])
                       & (cuts[:, j] - 1 >= cuts[:, j - 1]))
                cuts[:, j] -= can
                can2 = ((nW[:, j] == Sn[:, j])
                        & (nW[:, j - 1] + 1 < Sn[:, j - 1])
                        & (cuts[:, j] + 1 <= HI[:, j - 1])
                        & (cuts[:, j] + 1 <= cuts[:, j + 1]) & ~can)
                cuts[:, j] += can2
                moved += int(can.sum()) + int(can2.sum())
                nW = np.diff(cuts, axis=1)
            if moved == 0:
                break
        nW = np.diff(cuts, axis=1)
        if round_ == 2:
            break
        # re-sort tiles by the refined window profiles (compose permutations)
        for c in range(NC):
            prof = nW[c * NPC:(c + 1) * NPC]
            cand1 = np.lexsort((-prof[:, 3], -prof[:, 2], -prof[:, 1],
                                -prof[:, 0]))
            cand2 = _cluster(prof)
            best = (cand1 if _slots_of(cand1, prof) <= _slots_of(cand2, prof)
                    else cand2)
            orders[c] = orders[c][best]
        inv_orders = np.argsort(orders, axis=1)

    ncounts = nW.reshape(NC, NPC, NSUB)
    S = ncounts.reshape(NC, NT, 128, NSUB).max(axis=(0, 2))   # [NT, NSUB]

    # edges sorted by (node, src position)
    eorder = np.lexsort((spos, nid))
    s_spos = spos[eorder]
    run_starts = np.zeros(NC * NPC + 1, dtype=np.int64)
    np.cumsum(np.bincount(nid, minlength=NC * NPC), out=run_starts[1:])

    # group pairs of tiles into one gather per window: fewer, larger SWDGE
    # desc-gen instructions (the gen rate is the kernel's floor).
    GT = 1
    NG = NT // GT
    GW = np.zeros((NG, NSUB), dtype=np.int64)   # group window widths
    for g in range(NG):
        GW[g] = S[GT * g:GT * (g + 1)].sum(axis=0)
    gwt = GW.sum(axis=1)
    gidx_off = np.concatenate([[0], np.cumsum(8 * gwt)]).astype(np.int64)
    IDXW = int(gidx_off[-1])

    def _block(c, t, w):
        sq = int(S[t, w])
        nodes = c * NPC + t * 128 + np.arange(128)
        r0n = run_starts[nodes]
        lo = cuts[nodes, w]
        nq = nW[nodes, w]
        i = np.arange(sq)[:, None]
        mask = i < nq[None, :]
        gi_ = np.minimum(r0n[None, :] + lo[None, :] + i, len(s_spos) - 1)
        padw = PADPOS[w] - int(WBASE[w])
        return np.where(mask, s_spos[gi_] - int(WBASE[w]), padw)  # [sq, 128]

    gidx = np.zeros((NC, 128, IDXW), dtype=np.int16)
    for c in range(NC):
        for g in range(NG):
            parts = []
            for w in range(NSUB):
                for t in range(GT * g, GT * (g + 1)):
                    if int(S[t, w]):
                        parts.append(_block(c, t, w))
            flat = np.concatenate(parts, axis=0)       # [gwt[g], 128]
            w16 = flat.reshape(-1, 16).T.astype(np.int16)
            gidx[c, :, gidx_off[g]:gidx_off[g + 1]] = np.tile(w16, (8, 1))

    W1 = np.asarray(W1, f32); W2 = np.asarray(W2, f32)
    as1 = np.asarray(att_src1, f32); ad1 = np.asarray(att_dst1, f32)
    as2 = np.asarray(att_src2, f32); ad2 = np.asarray(att_dst2, f32)
    vs1 = np.stack([W1[:, h * C1:(h + 1) * C1] @ as1[h] for h in range(H1)], 1)
    vd1 = np.stack([W1[:, h * C1:(h + 1) * C1] @ ad1[h] for h in range(H1)], 1)
    wcat1 = np.concatenate([W1, vs1, vd1], axis=1).astype(f32)
    vs2 = (W2 @ as2[0])[:, None]
    vd2 = (W2 @ ad2[0])[:, None]
    wcat2 = np.concatenate([W2, vs2, vd2], axis=1).astype(f32)

    x = np.asarray(x, f32)
    xT = np.zeros((NC, F_IN, NPC), dtype=f32)
    glb = np.zeros((NC, NPC), dtype=np.int64)
    real = np.zeros((NC, NPC), dtype=bool)
    for c in range(NC):
        ol = orders[c]
        is_real = ol < NPC_REAL
        g = np.where(is_real, c * NPC_REAL + np.minimum(ol, NPC_REAL - 1), 0)
        xT[c] = np.where(is_real[:, None], x[g], 0.0).astype(f32).T
        glb[c] = g
        real[c] = is_real

    index = np.asarray(index, np.int64)
    seg = np.zeros((NC, NPC), dtype=np.int64)
    g0 = np.zeros(NC, dtype=np.int64)
    for c in range(NC):
        seg[c] = np.where(real[c], index[glb[c]], 0)
        s = seg[c][real[c]]
        g0[c] = s.min()
        assert s.max() - s.min() < WSEG, "segment window exceeds WSEG"
    f8 = ml_dtypes.float8_e4m3
    # ohf[c]: [NT*128, 256]  (lhsT chunks along free); oht[c]: [NT*128, 256]
    ohf = np.zeros((NC, NT * 128, WSEG), dtype=f8)
    oht = np.zeros((NC, NT * 128, WSEG), dtype=f8)
    for c in range(NC):
        for t in range(NT):
            sl = seg[c, t * 128:(t + 1) * 128] - g0[c]
            m = real[c, t * 128:(t + 1) * 128]
            oh = np.zeros((128, WSEG), dtype=np.float32)
            oh[np.arange(128)[m], sl[m]] = 1.0
            ohf[c, t * 128:(t + 1) * 128] = oh.astype(f8)
            # bwd lhsT chunk k: [128 segs, 128 nodes] -> store as [128, 2*128]
            ohtk = np.concatenate([oh[:, :128].T, oh[:, 128:].T], axis=1)
            oht[c, t * 128:(t + 1) * 128] = ohtk.astype(f8)

    padfix = np.zeros((128, 3), dtype=f32)
    padfix[84:128, :] = PAD_AS

    sidx = np.zeros((NC, 128, 2), dtype=np.int32)
    for c in range(NC):
        for k in range(2):
            sidx[c, :, k] = g0[c] + k * 128 + np.arange(128)

    b1t = np.tile(np.asarray(b1, f32)[None, :], (128, 1)).astype(f32)
    b2t = np.tile(np.asarray(b2, f32)[None, :], (128, 1)).astype(f32)

    per_core = [{
        "xT": np.ascontiguousarray(xT[c]),
        "wcat1": wcat1, "wcat2": wcat2, "b1t": b1t, "b2t": b2t,
        "gidx": np.ascontiguousarray(gidx[c]),
        "padfix": padfix,
        "ohf": np.ascontiguousarray(ohf[c]),
        "oht": np.ascontiguousarray(oht[c]),
        "sidx": np.ascontiguousarray(sidx[c]),
    } for c in range(NC)]
    shared = {"S": S, "GT": GT, "NG": NG, "GW": GW, "gidx_off": gidx_off,
              "IDXW": IDXW}
    asm = {"glb": glb, "real": real}
    return shared, per_core, asm


def _build(shared):
    import concourse.bass as bass
    import concourse.bacc as bacc
    import concourse.tile as tile
    from concourse import mybir, library_config
    from concourse.masks import make_identity

    S = shared["S"]; IDXW = shared["IDXW"]
    GT = shared["GT"]; NG = shared["NG"]; GW = shared["GW"]
    gidx_off = shared["gidx_off"]
    f32 = mybir.dt.float32
    bf16 = mybir.dt.bfloat16
    f8 = mybir.dt.float8e4
    i16 = mybir.dt.int16
    AL = mybir.AluOpType
    EXP = mybir.ActivationFunctionType.Exp
    COPYF = mybir.ActivationFunctionType.Copy
    RELU = mybir.ActivationFunctionType.Relu
    IOA = bass.IndirectOffsetOnAxis

    nc = bacc.Bacc("TRN2", target_bir_lowering=False, debug=False,
                   num_devices=NC, num_swdge_queues=4)

    xT_ext = nc.dram_tensor("xT", [F_IN, NPC], f32, kind="ExternalInput")
    wcat1_ext = nc.dram_tensor("wcat1", [F_IN, 36], f32, kind="ExternalInput")
    wcat2_ext = nc.dram_tensor("wcat2", [32, 18], f32, kind="ExternalInput")
    b1_ext = nc.dram_tensor("b1t", [128, 32], f32, kind="ExternalInput")
    b2_ext = nc.dram_tensor("b2t", [128, 16], f32, kind="ExternalInput")
    gidx_ext = nc.dram_tensor("gidx", [128, IDXW], i16, kind="ExternalInput")
    ohf_ext = nc.dram_tensor("ohf", [NT * 128, WSEG], f8, kind="ExternalInput")
    oht_ext = nc.dram_tensor("oht", [NT * 128, WSEG], f8, kind="ExternalInput")
    sidx_ext = nc.dram_tensor("sidx", [128, 2], mybir.dt.int32, kind="ExternalInput")
    padfix_ext = nc.dram_tensor("padfix", [128, 3], f32, kind="ExternalInput")
    out_ext = nc.dram_tensor("out", [NPC, OUT], f32, kind="ExternalOutput")

    with tile.TileContext(nc) as tc:
        with (
            tc.tile_pool(name="dram", bufs=1, space="DRAM") as dr,
            tc.tile_pool(name="const", bufs=1) as cpool,
            tc.tile_pool(name="sbuf", bufs=4) as sb,
            tc.tile_pool(name="gat", bufs=5) as gp,
            tc.tile_pool(name="gia", bufs=6) as gia,
            tc.tile_pool(name="psum", bufs=2, space="PSUM") as pp,
            tc.tile_pool(name="psum2", bufs=1, space="PSUM") as pp2,
            tc.tile_pool(name="psum_seg", bufs=1, space="PSUM") as pseg,
            tc.tile_pool(name="res", bufs=1) as rp,
        ):
            tab1_loc = dr.tile([NPC, TROW], f32, name="tab1_loc")
            tab2_loc = dr.tile([NPC, TROW], f32, name="tab2_loc")
            tab1_full = dr.tile([NC * NPC, TROW], f32, name="tab1_full",
                                addr_space="Shared")
            tab2_full = dr.tile([NC * NPC, TROW], f32, name="tab2_full",
                                addr_space="Shared")
            s_loc = dr.tile([1280, OUT], f32, name="s_loc")
            s_red = dr.tile([1280, OUT], f32, name="s_red", addr_space="Shared")

            nc.gpsimd.load_library(library_config.mlp)

            ident = cpool.tile([128, 128], f32, name="ident")
            make_identity(nc, ident[:])
            wc1 = cpool.tile([F_IN, 36], f32, name="wc1")
            nc.sync.dma_start(out=wc1[:], in_=wcat1_ext[:, :])
            wc2 = cpool.tile([32, 18], f32, name="wc2")
            nc.sync.dma_start(out=wc2[:], in_=wcat2_ext[:, :])
            b1s = cpool.tile([128, 32], f32, name="b1s")
            nc.sync.dma_start(out=b1s[:], in_=b1_ext[:, :])
            b2s = cpool.tile([128, 16], f32, name="b2s")
            nc.sync.dma_start(out=b2s[:], in_=b2_ext[:, :])
            pfx = cpool.tile([128, 3], f32, name="pfx")
            nc.sync.dma_start(out=pfx[:], in_=padfix_ext[:, :])

            ad1_all = rp.tile([128, NT * 2], f32, name="ad1_all")
            ad2_all = rp.tile([128, NT], f32, name="ad2_all")
            e_all = rp.tile([128, NT * OUT], f32, name="e_all")
            ebf_all = rp.tile([128, NT * OUT], bf16, name="ebf_all")
            x2_all = rp.tile([128, NT * 32], f32, name="x2_all")
            hs_all = rp.tile([128, NT * 36], f32, name="hs_all")
            h2s_all = rp.tile([128, NT * 18], f32, name="h2s_all")
            fo_all = rp.tile([128, NT * OUT], f32, name="fo_all")

            c02 = cpool.tile([128, 1], f32, name="c02")
            nc.vector.memset(c02[:], 0.2)
            c30 = cpool.tile([128, 1], f32, name="c30")
            nc.vector.memset(c30[:], 1e-30)

            # ---- phase 0: table1 rows -------------------------------------
            for t in range(NT):
                xt = sb.tile([128, 128], f32, name=f"xt{t}", tag="xt")
                nc.sync.dma_start(out=xt[:], in_=xT_ext[:, t * 128:(t + 1) * 128])
                hp = pp.tile([128, 36], f32, name=f"hp{t}", tag="hp")
                nc.tensor.matmul(out=hp[:], lhsT=xt[:], rhs=wc1[:],
                                 start=True, stop=True)
                hs = hs_all[:, 36 * t:36 * (t + 1)]
                nc.vector.tensor_copy(out=hs, in_=hp[:])
                if t == NT - 1:
                    nc.vector.tensor_tensor(out=hs_all[:, 36 * t + 32:36 * t + 34],
                                            in0=hs_all[:, 36 * t + 32:36 * t + 34],
                                            in1=pfx[:, 0:2], op=AL.add)
                nc.vector.tensor_copy(out=ad1_all[:, 2 * t:2 * t + 2],
                                      in_=hs_all[:, 36 * t + 34:36 * t + 36])
            nc.sync.dma_start(
                out=tab1_loc.rearrange("(t p) e -> p t e", p=128)[:, :, 0:36],
                in_=hs_all[:].rearrange("p (t e) -> p t e", e=36))

            nc.gpsimd.collective_compute(
                "AllGather", AL.bypass, replica_groups=[list(range(NC))],
                ins=[tab1_loc.opt()], outs=[tab1_full.opt()])

            # ---- phase 1 (+fused table2 rows), grouped gathers ------------
            for g in range(NG):
                gbase = int(gidx_off[g])
                gwtot = int(GW[g].sum())
                gi = gia.tile([128, 8 * gwtot], i16, name=f"gi1_{g}", tag="gi")
                nc.sync.dma_start(out=gi[:],
                                  in_=gidx_ext[:, gbase:gbase + 8 * gwtot])
                gqs = []
                colg = 0
                for q in range(NSUB):
                    gwq = int(GW[g, q])
                    if gwq == 0:
                        gqs.append(None)
                        continue
                    gq = gp.tile([128, gwq, TROW], f32, name=f"g1_{g}q{q}",
                                 tag=f"g1q{q}")
                    nidx = 128 * gwq
                    nc.gpsimd.dma_gather(
                        gq[:], tab1_full[WBASES[q]:WBASES[q] + 32768, :],
                        gi[:, 8 * colg:8 * (colg + gwq)],
                        nidx, nidx, TROW, queue_num=q, single_packet=False)
                    gqs.append(gq)
                    colg += gwq
                for j in range(GT):
                    t = GT * g + j
                    wt = int(S[t].sum())
                    gxs = []
                    for q in range(NSUB):
                        sq = int(S[t, q])
                        if sq == 0 or gqs[q] is None:
                            gxs.append((None, 0, 0))
                            continue
                        ofs = int(S[GT * g:t, q].sum())
                        gxs.append((gqs[q], sq, ofs))
                    x2 = sb.tile([128, 32], f32, name=f"x2_{t}", tag="x2")
                    for h in range(H1):
                        z = sb.tile([128, wt], f32, name=f"z{t}h{h}", tag=f"z{h}")
                        zs = sb.tile([128, wt], f32, name=f"zs{t}h{h}",
                                     tag=f"zs{h}")
                        adc = ad1_all[:, 2 * t + h:2 * t + h + 1]
                        adb = adc.to_broadcast([128, wt])
                        co = 0
                        for gq, sq, ofs in gxs:
                            if gq is None:
                                continue
                            a_s = gq[:, ofs:ofs + sq, 32 + h:33 + h].rearrange(
                                "p w e -> p (w e)")
                            nc.vector.tensor_tensor(out=z[:, co:co + sq],
                                                    in0=a_s,
                                                    in1=adb[:, co:co + sq],
                                                    op=AL.add)
                            co += sq
                        nc.vector.tensor_tensor(
                            out=zs[:], in0=z[:],
                            in1=c02[:, 0:1].to_broadcast([128, wt]), op=AL.mult)
                        nc.vector.tensor_tensor(out=z[:], in0=z[:], in1=zs[:],
                                                op=AL.max)
                        ex = sb.tile([128, wt], f32, name=f"ex{t}h{h}",
                                     tag=f"ex{h}")
                        den = sb.tile([128, 1], f32, name=f"den{t}h{h}",
                                      tag="den")
                        nc.scalar.activation(out=ex[:], in_=z[:], func=EXP,
                                             accum_out=den[:, 0:1])
                        msg = sb.tile([128, wt, C1], f32, name=f"msg{t}h{h}",
                                      tag="msg")
                        co = 0
                        for gq, sq, ofs in gxs:
                            if gq is None:
                                continue
                            exb = ex[:, co:co + sq, None].to_broadcast(
                                [128, sq, C1])
                            nc.vector.tensor_tensor(
                                out=msg[:, co:co + sq, :],
                                in0=gq[:, ofs:ofs + sq, 16 * h:16 * h + 16],
                                in1=exb, op=AL.mult)
                            co += sq
                        num = sb.tile([128, C1], f32, name=f"num{t}h{h}",
                                      tag="num")
                        nc.vector.reduce_sum(
                            out=num[:], in_=msg[:].rearrange("p w e -> p e w"),
                            axis=mybir.AxisListType.X)
                        nc.vector.tensor_tensor(out=den[:], in0=den[:],
                                                in1=c30[:, 0:1], op=AL.max)
                        rcp = sb.tile([128, 1], f32, name=f"rcp{t}h{h}",
                                      tag="rcp")
                        nc.vector.reciprocal(out=rcp[:], in_=den[:])
                        nc.vector.tensor_tensor(
                            out=x2[:, 16 * h:16 * h + 16], in0=num[:],
                            in1=rcp[:, 0:1].to_broadcast([128, C1]), op=AL.mult)
                    nc.vector.tensor_tensor(out=x2[:], in0=x2[:], in1=b1s[:],
                                            op=AL.add)
                    nc.scalar.activation(out=x2_all[:, 32 * t:32 * (t + 1)],
                                         in_=x2[:], func=RELU)
                    # fused table2 row for tile t
                    x2tp = pp2.tile([32, 128], f32, name=f"x2tp{t}", tag="x2tp")
                    nc.tensor.transpose(out=x2tp[:],
                                        in_=x2_all[:, 32 * t:32 * (t + 1)],
                                        identity=ident[:])
                    x2ts = sb.tile([32, 128], f32, name=f"x2ts{t}", tag="x2ts")
                    nc.scalar.activation(out=x2ts[:], in_=x2tp[:], func=COPYF)
                    h2p = pp2.tile([128, 18], f32, name=f"h2p{t}", tag="h2p")
                    nc.tensor.matmul(out=h2p[:], lhsT=x2ts[:], rhs=wc2[:],
                                     start=True, stop=True)
                    h2s = h2s_all[:, 18 * t:18 * (t + 1)]
                    nc.scalar.activation(out=h2s, in_=h2p[:], func=COPYF)
                    if t == NT - 1:
                        nc.vector.tensor_tensor(
                            out=h2s_all[:, 18 * t + 16:18 * t + 17],
                            in0=h2s_all[:, 18 * t + 16:18 * t + 17],
                            in1=pfx[:, 2:3], op=AL.add)
                    nc.scalar.activation(out=ad2_all[:, t:t + 1],
                                         in_=h2s_all[:, 18 * t + 17:18 * t + 18],
                                         func=COPYF)
            nc.sync.dma_start(
                out=tab2_loc.rearrange("(t p) e -> p t e", p=128)[:, :, 0:18],
                in_=h2s_all[:].rearrange("p (t e) -> p t e", e=18))

            nc.gpsimd.collective_compute(
                "AllGather", AL.bypass, replica_groups=[list(range(NC))],
                ins=[tab2_loc.opt()], outs=[tab2_full.opt()])

            # ---- phase 3: layer-2 aggregation + exp + segment partials ----
            sp = [pseg.tile([128, OUT], f32, name=f"segp{k}") for k in range(2)]
            for g in range(NG):
                gbase = int(gidx_off[g])
                gwtot = int(GW[g].sum())
                gi = gia.tile([128, 8 * gwtot], i16, name=f"gi2_{g}", tag="gi")
                nc.sync.dma_start(out=gi[:],
                                  in_=gidx_ext[:, gbase:gbase + 8 * gwtot])
                gqs = []
                colg = 0
                for q in range(NSUB):
                    gwq = int(GW[g, q])
                    if gwq == 0:
                        gqs.append(None)
                        continue
                    gq = gp.tile([128, gwq, TROW], f32, name=f"g2_{g}q{q}",
                                 tag=f"g1q{q}")
                    nidx = 128 * gwq
                    nc.gpsimd.dma_gather(
                        gq[:], tab2_full[WBASES[q]:WBASES[q] + 32768, :],
                        gi[:, 8 * colg:8 * (colg + gwq)],
                        nidx, nidx, TROW, queue_num=q, single_packet=False)
                    gqs.append(gq)
                    colg += gwq
                for j in range(GT):
                    t = GT * g + j
                    wt = int(S[t].sum())
                    gxs = []
                    for q in range(NSUB):
                        sq = int(S[t, q])
                        if sq == 0 or gqs[q] is None:
                            gxs.append((None, 0, 0))
                            continue
                        ofs = int(S[GT * g:t, q].sum())
                        gxs.append((gqs[q], sq, ofs))
                    z = sb.tile([128, wt], f32, name=f"z2_{t}", tag="z0")
                    zs = sb.tile([128, wt], f32, name=f"zs2_{t}", tag="zs0")
                    adc = ad2_all[:, t:t + 1]
                    adb = adc.to_broadcast([128, wt])
                    co = 0
                    for gq, sq, ofs in gxs:
                        if gq is None:
                            continue
                        a_s = gq[:, ofs:ofs + sq, 16:17].rearrange(
                            "p w e -> p (w e)")
                        nc.vector.tensor_tensor(out=z[:, co:co + sq], in0=a_s,
                                                in1=adb[:, co:co + sq],
                                                op=AL.add)
                        co += sq
                    nc.vector.tensor_tensor(
                        out=zs[:], in0=z[:],
                        in1=c02[:, 0:1].to_broadcast([128, wt]), op=AL.mult)
                    nc.vector.tensor_tensor(out=z[:], in0=z[:], in1=zs[:],
                                            op=AL.max)
                    ex = sb.tile([128, wt], f32, name=f"ex2_{t}", tag="ex0")
                    den = sb.tile([128, 1], f32, name=f"den2_{t}", tag="den")
                    nc.scalar.activation(out=ex[:], in_=z[:], func=EXP,
                                         accum_out=den[:, 0:1])
                    msg = sb.tile([128, wt, OUT], f32, name=f"msg2_{t}",
                                  tag="msg")
                    co = 0
                    for gq, sq, ofs in gxs:
                        if gq is None:
                            continue
                        exb = ex[:, co:co + sq, None].to_broadcast(
                            [128, sq, OUT])
                        nc.vector.tensor_tensor(out=msg[:, co:co + sq, :],
                                                in0=gq[:, ofs:ofs + sq, 0:16],
                                                in1=exb, op=AL.mult)
                        co += sq
                    num = sb.tile([128, OUT], f32, name=f"num2_{t}", tag="num")
                    nc.vector.reduce_sum(
                        out=num[:], in_=msg[:].rearrange("p w e -> p e w"),
                        axis=mybir.AxisListType.X)
                    nc.vector.tensor_tensor(out=den[:], in0=den[:],
                                            in1=c30[:, 0:1], op=AL.max)
                    rcp = sb.tile([128, 1], f32, name=f"rcp2_{t}", tag="rcp")
                    nc.vector.reciprocal(out=rcp[:], in_=den[:])
                    o2 = sb.tile([128, OUT], f32, name=f"o2_{t}", tag="o2")
                    nc.vector.tensor_tensor(
                        out=o2[:], in0=num[:],
                        in1=rcp[:, 0:1].to_broadcast([128, OUT]), op=AL.mult)
                    nc.vector.tensor_tensor(out=o2[:], in0=o2[:], in1=b2s[:],
                                            op=AL.add)
                    nc.scalar.activation(out=e_all[:, OUT * t:OUT * (t + 1)],
                                         in_=o2[:], func=EXP)
                    nc.scalar.activation(out=ebf_all[:, OUT * t:OUT * (t + 1)],
                                         in_=e_all[:, OUT * t:OUT * (t + 1)],
                                         func=COPYF)
                    ohf_t = sb.tile([128, WSEG], f8, name=f"ohf{t}", tag="ohf")
                    nc.sync.dma_start(out=ohf_t[:],
                                      in_=ohf_ext[t * 128:(t + 1) * 128, :])
                    for k in range(2):
                        nc.tensor.matmul(out=sp[k][:],
                                         lhsT=ohf_t[:, k * 128:(k + 1) * 128],
                                         rhs=ebf_all[:, OUT * t:OUT * (t + 1)],
                                         start=(t == 0), stop=(t == NT - 1))

            # ---- phase 4: combine segment sums across cores ---------------
            zt = sb.tile([128, 160], f32, name="zt")
            nc.vector.memset(zt[:], 0.0)
            nc.sync.dma_start(
                out=s_loc.rearrange("(c p) f -> p c f", p=128),
                in_=zt[:].rearrange("p (c f) -> p c f", c=10))
            sxi = sb.tile([128, 2], mybir.dt.int32, name="sxi")
            nc.sync.dma_start(out=sxi[:], in_=sidx_ext[:, :])
            for k in range(2):
                spc = sb.tile([128, OUT], f32, name=f"spc{k}", tag="spc")
                nc.vector.tensor_copy(out=spc[:], in_=sp[k][:])
                nc.gpsimd.indirect_dma_start(
                    out=s_loc[:, :],
                    out_offset=IOA(ap=sxi[:, k:k + 1], axis=0),
                    in_=spc[:], in_offset=None)

            nc.gpsimd.collective_compute(
                "AllReduce", AL.add, replica_groups=[list(range(NC))],
                ins=[s_loc.opt()], outs=[s_red.opt()])

            sw = []
            for k in range(2):
                swf = sb.tile([128, OUT], f32, name=f"swf{k}", tag="swf")
                nc.gpsimd.indirect_dma_start(
                    out=swf[:], out_offset=None,
                    in_=s_red[:, :],
                    in_offset=IOA(ap=sxi[:, k:k + 1], axis=0))
                swb = rp.tile([128, OUT], bf16, name=f"sw{k}")
                nc.vector.tensor_copy(out=swb[:], in_=swf[:])
                sw.append(swb)

            # ---- phase 5: divide, write out -------------------------------
            for t in range(NT):
                oht_t = sb.tile([128, WSEG], f8, name=f"oht{t}", tag="oht")
                nc.sync.dma_start(out=oht_t[:],
                                  in_=oht_ext[t * 128:(t + 1) * 128, :])
                dp = pp.tile([128, OUT], f32, name=f"dp{t}", tag="dp")
                for k in range(2):
                    nc.tensor.matmul(out=dp[:],
                                     lhsT=oht_t[:, k * 128:(k + 1) * 128],
                                     rhs=sw[k][:], start=(k == 0), stop=(k == 1))
                dd = sb.tile([128, OUT], f32, name=f"dd{t}", tag="dd")
                nc.vector.tensor_tensor(out=dd[:], in0=dp[:],
                                        in1=c30[:, 0:1].to_broadcast([128, OUT]),
                                        op=AL.max)
                nc.vector.reciprocal(out=dd[:], in_=dd[:])
                nc.vector.tensor_tensor(out=fo_all[:, OUT * t:OUT * (t + 1)],
                                        in0=e_all[:, OUT * t:OUT * (t + 1)],
                                        in1=dd[:], op=AL.mult)
            nc.sync.dma_start(
                out=out_ext.rearrange("(t p) e -> p t e", p=128),
                in_=fo_all[:].rearrange("p (t e) -> p t e", e=OUT))

    nc.compile()
    return nc


def kernel_impl(inputs, trace=False):
    from concourse.bass_utils import run_bass_kernel_spmd
    shared, per_core, asm = _preprocess(**inputs)
    nc = _build(shared)
    res = run_bass_kernel_spmd(nc, per_core, core_ids=list(range(NC)),
                               trace=trace)
    out = np.zeros((N, OUT), dtype=np.float32)
    for c in range(NC):
        o = np.asarray(res.results[c]["out"])
        m = asm["real"][c]
        out[asm["glb"][c][m]] = o[m]
    return out, res


def kernel(**inputs):
    out, _ = kernel_impl(inputs, trace=False)
    return out

